# revision 31
# baseline (speedup 1.0000x reference)
"""Trainium2 Bass kernel for nn_AttentionLayer (scatter_memory).

Reference math (per batch b):
    heatmap[k,y,x] += vis_k at (y_k, x_k)              # scatter, <=19 nonzero px
    kp_feat = conv1x1_K->K(heatmap)                    # kp_proj_w/b
    img_proj = img_fc(img)                             # C x C linear over pixels
    kp_proj  = kp_fc(kp_feat)                          # K -> C linear
    combined = tanh(img_proj + kp_proj)
    scores   = sigmoid(attn_fc(combined))              # per-pixel scalar
    out      = img * scores

The keypoint path only perturbs the <=19 pixel columns hit by a keypoint:
    pre[o,s] = W img[:,s] + b_total + sum_{j: s_j == s} vis_j M[:,j]
with W = img_fc_w, M = kp_fc_w @ kp_proj_w, b_total folded on host.  The
device computes the DENSE no-keypoint path for all 16384 pixels, plus a tiny
19-column "fixup" using host-gathered image columns and a host-built [19,19]
collision matrix G[j',j] = vis_j' * (s_j' == s_j); the host overwrites those
<=19 columns of the returned image with the fixup columns (index math on host
is exact: /128 is a power-of-two divide).

Memory regime: all image I/O is bf16 (host casts in, host upcasts out), which
halves HBM traffic to ~16.8 MB/core (~47 us at 358 GB/s/NC).  1 MB DMA chunks,
all on the sync HWDGE ring so the Act engine runs activations only.

Software pipeline, 3 stages deep (per 1024-px step p):
  PE : attn-reduce(p-3) [2 ones-matmuls], then 8 main matmuls(p)
  Act: sigmoid(p-3), then tanh x2 (p-1)  -- each one wide [128,1024] op over a
       2-bank PSUM tile
  DVE: scores-multiply x2 (p-3), then a*tanh weighted-sum x2 (p-1)
The attention z = sum_c a_c * comb[c,s] is computed as two per-partition-scalar
DVE FMAs (a broadcast lives in a [128,1] column) followed by a ones-weights
matmul, whose PSUM result is already broadcast across all 128 partitions, so
sigmoid and the final multiply need no partition-broadcast step.

Sharding: pure data parallelism, batch b -> NeuronCore b (weights replicated).
"""

import sys
from collections import deque
from contextlib import ExitStack

import numpy as np

sys.path.insert(0, "/opt/trn_rl_repo")

import concourse.bacc as bacc
import concourse.bass as bass
import concourse.mybir as mybir
import concourse.tile as tile
from concourse.bass_utils import run_bass_kernel_spmd

F32 = mybir.dt.float32
BF16 = mybir.dt.bfloat16
AF = mybir.ActivationFunctionType
OP = mybir.AluOpType

B, C, H, W, K = 8, 256, 128, 128, 19
S = H * W                  # 16384 pixels
PT = 1024                  # pixels per pipeline step (2 PSUM banks of f32)
NP = S // PT               # 16 steps
CH = 4096                  # pixels per DMA chunk (1 MB bf16 per half)
PPC = CH // PT             # 4 steps per chunk
NCH = S // CH              # 4 chunks
_CACHE: dict = {}


def _emit(tc: tile.TileContext, io: dict):
    nc = tc.nc
    img, imgcb, gb, wt, mt, bias, acol, ab, out, ofix = (
        io["img"], io["imgcb"], io["gb"], io["wt"], io["mt"],
        io["bias"], io["acol"], io["ab"], io["out"], io["ofix"],
    )
    with ExitStack() as ctx:
        consts = ctx.enter_context(tc.tile_pool(name="consts", bufs=1))
        imgp = ctx.enter_context(tc.tile_pool(name="imgp", bufs=1))
        outp = ctx.enter_context(tc.tile_pool(name="outp", bufs=3))
        combp = ctx.enter_context(tc.tile_pool(name="combp", bufs=2))
        cbsp = ctx.enter_context(tc.tile_pool(name="cbsp", bufs=3))
        scorep = ctx.enter_context(tc.tile_pool(name="scorep", bufs=2))
        psum = ctx.enter_context(tc.tile_pool(name="psum", bufs=1, space="PSUM"))

        # chunk layout: small chunks first so compute starts early, then
        # 4096-px chunks for DMA efficiency
        CHOFF = [0, 1024, 2048, 4096, 8192, 12288]
        CHSZ = [1024, 1024, 2048, 4096, 4096, 4096]
        P2C = [0, 1, 2, 2] + [3] * 4 + [4] * 4 + [5] * 4   # pair -> chunk
        im0s, im1s, oc0s, oc1s = [], [], [], []

        def load_chunk(c):
            im0 = imgp.tile([128, CHSZ[c]], BF16, tag=f"im0_{c}", name=f"im0_{c}")
            im1 = imgp.tile([128, CHSZ[c]], BF16, tag=f"im1_{c}", name=f"im1_{c}")
            csl = slice(CHOFF[c], CHOFF[c] + CHSZ[c])
            nc.sync.dma_start(im0[:], img[0:128, csl])
            nc.sync.dma_start(im1[:], img[128:256, csl])
            im0s.append(im0)
            im1s.append(im1)

        # ---- constants into SBUF, ordered so chunk0 lands ASAP ----
        wt0 = consts.tile([128, C], BF16)          # W^T rows c=0..127
        wt1 = consts.tile([128, C], BF16)          # W^T rows c=128..255
        nc.sync.dma_start(wt0[:], wt[0:128, :])
        nc.sync.dma_start(wt1[:], wt[128:256, :])
        b0 = consts.tile([128, 1], F32)
        b1 = consts.tile([128, 1], F32)
        nc.sync.dma_start(b0[:], bias[0:128, :])
        nc.sync.dma_start(b1[:], bias[128:256, :])
        abt = consts.tile([128, 1], F32)
        nc.sync.dma_start(abt[:], ab[:, :])
        a0c = consts.tile([128, 1], F32)           # attn_fc_w as per-partition
        a1c = consts.tile([128, 1], F32)
        nc.sync.dma_start(a0c[:], acol[0:128, :])
        nc.sync.dma_start(a1c[:], acol[128:256, :])
        ones = consts.tile([128, 128], BF16)       # partition-sum stationary
        nc.vector.memset(ones[:], 1.0)
        load_chunk(0)
        load_chunk(1)
        load_chunk(2)
        # fixup constants (consumed by the pre-loop fixup)
        mts = consts.tile([K, C], BF16)            # M^T [19, 256]
        nc.sync.dma_start(mts[:], mt[:, :])
        gbt = consts.tile([K, K], BF16)            # collision matrix
        nc.sync.dma_start(gbt[:], gb[:, :])
        ic0b = consts.tile([128, K], BF16)         # img cols
        ic1b = consts.tile([128, K], BF16)
        nc.sync.dma_start(ic0b[:], imgcb[0:128, :])
        nc.sync.dma_start(ic1b[:], imgcb[128:256, :])

        h0, h1 = bass.ts(0, 512), bass.ts(1, 512)
        kk = bass.ts(0, K)
        pres, cbss, scs = {}, {}, {}
        fix = {}                   # keypoint-fixup tiles, built mid-loop

        def ib_sl(pd):
            c = P2C[pd]
            off = pd * PT - CHOFF[c]
            return im0s[c][:, off:off + PT], im1s[c][:, off:off + PT]

        def emit_main(p):
            ib0, ib1 = ib_sl(p)
            pre0 = psum.tile([128, PT], F32, tag="pre", bufs=3, name="pre0")
            pre1 = psum.tile([128, PT], F32, tag="pre", bufs=3, name="pre1")
            nc.tensor.matmul(out=pre0[:, h0], lhsT=wt0[:, 0:128], rhs=ib0[:, h0], start=True, stop=False)
            nc.tensor.matmul(out=pre0[:, h1], lhsT=wt0[:, 0:128], rhs=ib0[:, h1], start=True, stop=False)
            nc.tensor.matmul(out=pre0[:, h0], lhsT=wt1[:, 0:128], rhs=ib1[:, h0], start=False, stop=True)
            nc.tensor.matmul(out=pre0[:, h1], lhsT=wt1[:, 0:128], rhs=ib1[:, h1], start=False, stop=True)
            nc.tensor.matmul(out=pre1[:, h0], lhsT=wt0[:, 128:256], rhs=ib0[:, h0], start=True, stop=False)
            nc.tensor.matmul(out=pre1[:, h1], lhsT=wt0[:, 128:256], rhs=ib0[:, h1], start=True, stop=False)
            nc.tensor.matmul(out=pre1[:, h0], lhsT=wt1[:, 128:256], rhs=ib1[:, h0], start=False, stop=True)
            nc.tensor.matmul(out=pre1[:, h1], lhsT=wt1[:, 128:256], rhs=ib1[:, h1], start=False, stop=True)
            pres[p] = (pre0, pre1)

        def emit_attn(p):
            cbs = cbss.pop(p)
            # the last attns run after the main matmuls are done, so they can
            # borrow freed "pre" psum banks -- avoids serializing the tail on
            # the single pz buffer
            if p >= NP - 2:
                pz = psum.tile([128, PT], F32, tag="pre", bufs=3, name="pzt")
            else:
                pz = psum.tile([128, PT], F32, tag="pz", bufs=1, name="pz")
            nc.tensor.matmul(out=pz[:, h0], lhsT=ones[:], rhs=cbs[:, h0], start=True, stop=True)
            nc.tensor.matmul(out=pz[:, h1], lhsT=ones[:], rhs=cbs[:, h1], start=True, stop=True)
            return pz

        def emit_tanh(p):
            pre0, pre1 = pres.pop(p)
            cb0 = combp.tile([128, PT], BF16, tag="cb0", name="cb0")
            cb1 = combp.tile([128, PT], BF16, tag="cb1", name="cb1")
            nc.scalar.activation(cb0[:], pre0[:], AF.Tanh, bias=b0[:, 0:1])
            nc.scalar.activation(cb1[:], pre1[:], AF.Tanh, bias=b1[:, 0:1])
            # cbs = a0*cb0 + a1*cb1  (per-partition scalars; z = ones^T cbs)
            cbt = cbsp.tile([128, PT], BF16, tag="cbt", bufs=2, name="cbt")
            nc.vector.tensor_scalar(cbt[:], cb0[:], a0c[:, 0:1], None, OP.mult)
            cbu = cbsp.tile([128, PT], BF16, tag="cbu", bufs=2, name="cbu")
            nc.vector.tensor_scalar(cbu[:], cb1[:], a1c[:, 0:1], None, OP.mult)
            cbs = cbsp.tile([128, PT], BF16, tag="cbs", bufs=4, name="cbs")
            nc.vector.tensor_tensor(cbs[:], cbt[:], cbu[:], op=OP.add)
            cbss[p] = cbs

        def emit_sigmoid(p, pz):
            sc = scorep.tile([128, PT], BF16, tag="sc", name="sc")
            nc.scalar.activation(sc[:], pz[:], AF.Sigmoid, bias=abt[:, 0:1])
            scs[p] = sc

        def emit_mul(pd):
            sc = scs.pop(pd)
            ib0, ib1 = ib_sl(pd)
            if pd % 2 == 0:
                o0 = outp.tile([128, 2 * PT], BF16, tag="oc0", name="o0")
                o1 = outp.tile([128, 2 * PT], BF16, tag="oc1", name="o1")
                oc0s.append(o0)
                oc1s.append(o1)
            wsl = slice((pd % 2) * PT, (pd % 2) * PT + PT)
            nc.vector.tensor_mul(oc0s[-1][:, wsl], ib0[:], sc[:])
            nc.vector.tensor_mul(oc1s[-1][:, wsl], ib1[:], sc[:])
            if pd >= NP - 2:
                # drain the last pairs as soon as each is multiplied
                ssl = bass.ts(pd, PT)
                nc.sync.dma_start(out[0:128, ssl], oc0s[-1][:, wsl])
                nc.sync.dma_start(out[128:256, ssl], oc1s[-1][:, wsl])
            elif pd % 2 == 1:
                ssl = bass.ts(pd // 2, 2 * PT)
                nc.sync.dma_start(out[0:128, ssl], oc0s[-1][:])
                nc.sync.dma_start(out[128:256, ssl], oc1s[-1][:])

        def fixup_part1():
            # pre-tanh + tanh + a-weighting for the <=19 keypoint columns.
            # Both 128-channel halves live in ONE pz-tag psum tile: cols
            # [0:19] in the first bank, [512:531] in the second, so each is
            # its own accumulation group and no extra psum bank is needed.
            pf = psum.tile([128, PT], F32, tag="pz", bufs=1, name="pf")
            kkB = slice(512, 512 + K)
            nc.tensor.matmul(out=pf[:, kk], lhsT=wt0[:, 0:128], rhs=ic0b[:], start=True, stop=False)
            nc.tensor.matmul(out=pf[:, kk], lhsT=wt1[:, 0:128], rhs=ic1b[:], start=False, stop=False)
            nc.tensor.matmul(out=pf[:, kk], lhsT=mts[:, 0:128], rhs=gbt[:], start=False, stop=True)
            nc.tensor.matmul(out=pf[:, kkB], lhsT=wt0[:, 128:256], rhs=ic0b[:], start=True, stop=False)
            nc.tensor.matmul(out=pf[:, kkB], lhsT=wt1[:, 128:256], rhs=ic1b[:], start=False, stop=False)
            nc.tensor.matmul(out=pf[:, kkB], lhsT=mts[:, 128:256], rhs=gbt[:], start=False, stop=True)
            cf0 = consts.tile([128, K], BF16)
            cf1 = consts.tile([128, K], BF16)
            nc.scalar.activation(cf0[:], pf[:, kk], AF.Tanh, bias=b0[:, 0:1])
            nc.scalar.activation(cf1[:], pf[:, kkB], AF.Tanh, bias=b1[:, 0:1])
            cft = consts.tile([128, K], BF16)
            nc.vector.tensor_scalar(cft[:], cf0[:], a0c[:, 0:1], None, OP.mult)
            cfs = consts.tile([128, K], BF16)
            nc.vector.scalar_tensor_tensor(
                cfs[:], cf1[:], a1c[:, 0:1], cft[:], op0=OP.mult, op1=OP.add)
            fix["cfs"] = cfs

        def fixup_part2():
            pzf = psum.tile([128, PT], F32, tag="pz", bufs=1, name="pzf")
            nc.tensor.matmul(out=pzf[:, kk], lhsT=ones[:], rhs=fix["cfs"][:], start=True, stop=True)
            scf = consts.tile([128, K], F32)
            nc.scalar.activation(scf[:], pzf[:, kk], AF.Sigmoid, bias=abt[:, 0:1])
            of0 = consts.tile([128, K], F32)
            of1 = consts.tile([128, K], F32)
            nc.vector.tensor_mul(of0[:], ic0b[:], scf[:])
            nc.vector.tensor_mul(of1[:], ic1b[:], scf[:])
            nc.sync.dma_start(ofix[0:128, :], of0[:])
            nc.sync.dma_start(ofix[128:256, :], of1[:])

        # PE warm-up: dummy matmuls on the weight tiles run while chunk0 is
        # still in flight, ramping the tensor engine out of its low p-state
        # (full clock needs ~3us of continuous execution) and priming FWL.
        for i in range(5):
            wp = psum.tile([128, PT], F32, tag="pre", bufs=3, name=f"wp{i}")
            nc.tensor.matmul(out=wp[:, 0:256], lhsT=wt0[:, 0:128], rhs=wt1[:], start=True, stop=True)
            nc.tensor.matmul(out=wp[:, 256:512], lhsT=wt1[:, 0:128], rhs=wt0[:], start=True, stop=True)

        # stage lags: main(p) / tanh(p-1) / attn+sigmoid(p-2) / mul+store(p-3)
        # -- ordered so every engine's queue head is ready (or nearly so) at
        # iteration start.
        CHUNK_AT = {0: 3, 4: 4, 8: 5}      # prefetch schedule (ch0-2 upfront)
        for p in range(NP + 3):
            if p in CHUNK_AT:
                load_chunk(CHUNK_AT[p])
            if p - 2 >= 0 and p - 2 < NP:
                pz = emit_attn(p - 2)
            if p < NP:
                emit_main(p)
            if p - 3 >= 0 and p - 3 < NP:
                emit_mul(p - 3)
            if p - 1 >= 0 and p - 1 < NP:
                emit_tanh(p - 1)
            if p - 2 >= 0 and p - 2 < NP:
                emit_sigmoid(p - 2, pz)
            # keypoint fixup interleaves with the tail iterations: its tiny
            # matmul/act/mul chain overlaps the final stores
            if p == NP:
                fixup_part1()
            elif p == NP + 1:
                fixup_part2()


def _build():
    if "nc" in _CACHE:
        return _CACHE["nc"]
    nc = bacc.Bacc("TRN2", target_bir_lowering=False, debug=False)
    io = {
        "img": nc.dram_tensor("img", [C, S], BF16, kind="ExternalInput").ap(),
        "imgcb": nc.dram_tensor("imgcb", [C, K], BF16, kind="ExternalInput").ap(),
        "gb": nc.dram_tensor("gb", [K, K], BF16, kind="ExternalInput").ap(),
        "wt": nc.dram_tensor("wt", [C, C], BF16, kind="ExternalInput").ap(),
        "mt": nc.dram_tensor("mt", [K, C], BF16, kind="ExternalInput").ap(),
        "bias": nc.dram_tensor("bias", [C, 1], F32, kind="ExternalInput").ap(),
        "acol": nc.dram_tensor("acol", [C, 1], F32, kind="ExternalInput").ap(),
        "ab": nc.dram_tensor("ab", [128, 1], F32, kind="ExternalInput").ap(),
        "out": nc.dram_tensor("out", [C, S], BF16, kind="ExternalOutput").ap(),
        "ofix": nc.dram_tensor("ofix", [C, K], F32, kind="ExternalOutput").ap(),
    }
    with tile.TileContext(nc) as tc:
        _emit(tc, io)
    nc.compile()
    _CACHE["nc"] = nc
    return nc


def _host_indices(keypoint_features):
    """Exact replication of the reference index math (all ops are exact in
    fp32: /128 is a power-of-two divide, clip, truncate)."""
    kps = np.asarray(keypoint_features, dtype=np.float32)        # [B, K, 3]
    x = np.clip(kps[:, :, 0] / np.float32(W), 0.0, W - 1).astype(np.int32)
    y = np.clip(kps[:, :, 1] / np.float32(H), 0.0, H - 1).astype(np.int32)
    s = y.astype(np.int64) * W + x                                # [B, K]
    vis = kps[:, :, 2] > 0                                        # [B, K]
    return s, vis


def _in_maps(image_features, keypoint_features, img_fc_w, img_fc_b,
             kp_proj_w, kp_proj_b, kp_fc_w, kp_fc_b, attn_fc_w, attn_fc_b):
    import ml_dtypes

    f = lambda a: np.ascontiguousarray(np.asarray(a, dtype=np.float32))
    bf = lambda a: np.ascontiguousarray(
        np.asarray(a, dtype=np.float32).astype(ml_dtypes.bfloat16))
    img_fc_w, img_fc_b = f(img_fc_w), f(img_fc_b)
    kp_proj_w, kp_proj_b = f(kp_proj_w), f(kp_proj_b)
    kp_fc_w, kp_fc_b = f(kp_fc_w), f(kp_fc_b)
    attn_fc_w, attn_fc_b = f(attn_fc_w), f(attn_fc_b)

    wt = bf(img_fc_w.T)                                         # [C, C]
    mt = bf((kp_fc_w @ kp_proj_w).T)                            # [K, C]
    bias = f((img_fc_b + kp_fc_w @ kp_proj_b + kp_fc_b).reshape(C, 1))
    acol = f(attn_fc_w.reshape(C, 1))
    ab = np.full((128, 1), float(attn_fc_b.reshape(-1)[0]), np.float32)

    imgs = f(image_features).reshape(B, C, S)
    s, vis = _host_indices(keypoint_features)
    maps = []
    for b in range(B):
        g = (s[b][None, :] == s[b][:, None]) & vis[b][:, None]  # [j', j]
        imgc = np.ascontiguousarray(imgs[b][:, s[b]])           # [C, K]
        maps.append({
            "img": bf(imgs[b]),
            "imgcb": bf(imgc),
            "gb": bf(g.astype(np.float32)),
            "wt": wt, "mt": mt, "bias": bias, "acol": acol, "ab": ab,
        })
    return maps


def _run(in_maps, trace=False, tmpdir=None):
    nc = _build()
    return run_bass_kernel_spmd(
        nc, in_maps, core_ids=list(range(B)), trace=trace, tmpdir=tmpdir
    )


def _assemble(res, keypoint_features):
    s, _ = _host_indices(keypoint_features)
    outs = []
    for b in range(B):
        o = np.asarray(res.results[b]["out"]).astype(np.float32)  # [C, S]
        o[:, s[b]] = np.asarray(res.results[b]["ofix"])           # fixup cols
        outs.append(o.reshape(C, H, W))
    return np.stack(outs)


def kernel(**inputs) -> np.ndarray:
    res = _run(_in_maps(**inputs))
    return _assemble(res, inputs["keypoint_features"])


def _enable_axon_ntff_hook():
    """Recreate the missing antenv.axon_hooks module and register the NTFF
    profile hook (what trn_boot would do if the image shipped axon_hooks).
    Local profiling only; kernel() never calls this."""
    import types

    if "antenv.axon_hooks" in sys.modules:
        return
    mod = types.ModuleType("antenv.axon_hooks")
    state = {"hook": None}
    mod.set_axon_ntff_profile_hook = lambda h: state.__setitem__("hook", h)
    mod.get_axon_ntff_profile_hook = lambda: state["hook"]
    sys.modules["antenv.axon_hooks"] = mod
    import antenv

    antenv.axon_hooks = mod
    from trn_agent_boot.trn_boot import _ntff_profile_via_ctypes

    mod.set_axon_ntff_profile_hook(_ntff_profile_via_ctypes("/opt/axon/libaxon_pjrt.so"))
    # keep artifacts local -- no bucket in this container
    import concourse.bass_utils as bu

    bu.upload_artifacts = lambda tmpdir: tmpdir


def kernel_traced(**inputs):
    """Like kernel() but profiles: returns (out, exec_time_ns, tmpdir)."""
    import tempfile

    _enable_axon_ntff_hook()
    tmpdir = tempfile.mkdtemp(prefix="bass_trace_")
    res = _run(_in_maps(**inputs), trace=True, tmpdir=tmpdir)
    out = _assemble(res, inputs["keypoint_features"])
    return out, res.exec_time_ns, tmpdir


# revision 32
# speedup vs baseline: 1.1710x; 1.1710x over previous
"""Trainium2 Bass kernel for nn_AttentionLayer (scatter_memory).

Reference math (per batch b):
    heatmap[k,y,x] += vis_k at (y_k, x_k)              # scatter, <=19 nonzero px
    kp_feat = conv1x1_K->K(heatmap)                    # kp_proj_w/b
    img_proj = img_fc(img)                             # C x C linear over pixels
    kp_proj  = kp_fc(kp_feat)                          # K -> C linear
    combined = tanh(img_proj + kp_proj)
    scores   = sigmoid(attn_fc(combined))              # per-pixel scalar
    out      = img * scores

The keypoint path only perturbs the <=19 pixel columns hit by a keypoint:
    pre[o,s] = W img[:,s] + b_total + sum_{j: s_j == s} vis_j M[:,j]
with W = img_fc_w, M = kp_fc_w @ kp_proj_w, b_total folded on host.  The
device computes the DENSE no-keypoint path for all 16384 pixels, plus a tiny
19-column "fixup" using host-gathered image columns and a host-built [19,19]
collision matrix G[j',j] = vis_j' * (s_j' == s_j); the host overwrites those
<=19 columns of the returned image with the fixup columns (index math on host
is exact: /128 is a power-of-two divide).

Memory regime: all image I/O is bf16 (host casts in, host upcasts out), which
halves HBM traffic to ~16.8 MB/core (~47 us at 358 GB/s/NC).  Image loads and
output stores all ride the sync HWDGE ring (ordered so small constants and the
growing-size first chunks land first), leaving the Act engine to run
activations only -- Act is the steady-state pacer at 3 wide [128,1024] ops
(2 tanh + 1 sigmoid ~= 3.0 us) per 1024-px step.

Software pipeline (per 1024-px step p), each engine's queue head ready at
iteration start:
  PE : attn-reduce(p-2) [2 ones-matmuls], then 8 main matmuls(p)
       (plus ~10 warm-up matmuls pre-loop to ramp the PE p-state while the
       first image chunk is in flight)
  Act: tanh x2 (p-1) over 2-bank PSUM tiles, then sigmoid(p-2)
  DVE: scores-multiply x2 (p-3), then the a*tanh weighted-sum (p-1)
The attention z = sum_c a_c * comb[c,s] is computed as two per-partition-scalar
DVE multiplies (a lives in a [128,1] column) + add, then a ones-weights matmul
whose PSUM result is already broadcast across all 128 partitions, so sigmoid
and the final multiply need no partition-broadcast step.  The keypoint fixup
is emitted into the pipeline-drain iterations where all engines have slack.

Sharding: pure data parallelism, batch b -> NeuronCore b (weights replicated).
"""

import sys
from collections import deque
from contextlib import ExitStack

import numpy as np

sys.path.insert(0, "/opt/trn_rl_repo")

import concourse.bacc as bacc
import concourse.bass as bass
import concourse.mybir as mybir
import concourse.tile as tile
from concourse.bass_utils import run_bass_kernel_spmd

F32 = mybir.dt.float32
BF16 = mybir.dt.bfloat16
AF = mybir.ActivationFunctionType
OP = mybir.AluOpType

B, C, H, W, K = 8, 256, 128, 128, 19
S = H * W                  # 16384 pixels
PT = 1024                  # pixels per pipeline step (2 PSUM banks of f32)
NP = S // PT               # 16 steps
CH = 4096                  # pixels per DMA chunk (1 MB bf16 per half)
PPC = CH // PT             # 4 steps per chunk
NCH = S // CH              # 4 chunks
_CACHE: dict = {}


def _emit(tc: tile.TileContext, io: dict):
    nc = tc.nc
    img, imgcb, gb, wt, mt, bias, acol, ab, out, ofix = (
        io["img"], io["imgcb"], io["gb"], io["wt"], io["mt"],
        io["bias"], io["acol"], io["ab"], io["out"], io["ofix"],
    )
    with ExitStack() as ctx:
        consts = ctx.enter_context(tc.tile_pool(name="consts", bufs=1))
        imgp = ctx.enter_context(tc.tile_pool(name="imgp", bufs=1))
        outp = ctx.enter_context(tc.tile_pool(name="outp", bufs=3))
        combp = ctx.enter_context(tc.tile_pool(name="combp", bufs=2))
        cbsp = ctx.enter_context(tc.tile_pool(name="cbsp", bufs=3))
        scorep = ctx.enter_context(tc.tile_pool(name="scorep", bufs=2))
        psum = ctx.enter_context(tc.tile_pool(name="psum", bufs=1, space="PSUM"))

        # chunk layout: small chunks first so compute starts early, then
        # 4096-px chunks for DMA efficiency
        CHOFF = [0, 1024, 2048, 4096, 8192, 12288]
        CHSZ = [1024, 1024, 2048, 4096, 4096, 4096]
        P2C = [0, 1, 2, 2] + [3] * 4 + [4] * 4 + [5] * 4   # pair -> chunk
        im0s, im1s, oc0s, oc1s = [], [], [], []

        def load_chunk(c):
            im0 = imgp.tile([128, CHSZ[c]], BF16, tag=f"im0_{c}", name=f"im0_{c}")
            im1 = imgp.tile([128, CHSZ[c]], BF16, tag=f"im1_{c}", name=f"im1_{c}")
            csl = slice(CHOFF[c], CHOFF[c] + CHSZ[c])
            nc.sync.dma_start(im0[:], img[0:128, csl])
            nc.sync.dma_start(im1[:], img[128:256, csl])
            im0s.append(im0)
            im1s.append(im1)

        # ---- constants into SBUF, ordered so chunk0 lands ASAP ----
        wt0 = consts.tile([128, C], BF16)          # W^T rows c=0..127
        wt1 = consts.tile([128, C], BF16)          # W^T rows c=128..255
        nc.sync.dma_start(wt0[:], wt[0:128, :])
        nc.sync.dma_start(wt1[:], wt[128:256, :])
        b0 = consts.tile([128, 1], F32)
        b1 = consts.tile([128, 1], F32)
        nc.sync.dma_start(b0[:], bias[0:128, :])
        nc.sync.dma_start(b1[:], bias[128:256, :])
        abt = consts.tile([128, 1], F32)
        nc.sync.dma_start(abt[:], ab[:, :])
        a0c = consts.tile([128, 1], F32)           # attn_fc_w as per-partition
        a1c = consts.tile([128, 1], F32)
        nc.sync.dma_start(a0c[:], acol[0:128, :])
        nc.sync.dma_start(a1c[:], acol[128:256, :])
        ones = consts.tile([128, 128], BF16)       # partition-sum stationary
        nc.vector.memset(ones[:], 1.0)
        load_chunk(0)
        load_chunk(1)
        load_chunk(2)
        # fixup constants (consumed by the pre-loop fixup)
        mts = consts.tile([K, C], BF16)            # M^T [19, 256]
        nc.sync.dma_start(mts[:], mt[:, :])
        gbt = consts.tile([K, K], BF16)            # collision matrix
        nc.sync.dma_start(gbt[:], gb[:, :])
        ic0b = consts.tile([128, K], BF16)         # img cols
        ic1b = consts.tile([128, K], BF16)
        nc.sync.dma_start(ic0b[:], imgcb[0:128, :])
        nc.sync.dma_start(ic1b[:], imgcb[128:256, :])

        h0, h1 = bass.ts(0, 512), bass.ts(1, 512)
        kk = bass.ts(0, K)
        pres, cbss, scs = {}, {}, {}
        fix = {}                   # keypoint-fixup tiles, built mid-loop

        def ib_sl(pd):
            c = P2C[pd]
            off = pd * PT - CHOFF[c]
            return im0s[c][:, off:off + PT], im1s[c][:, off:off + PT]

        def emit_main(p):
            ib0, ib1 = ib_sl(p)
            pre0 = psum.tile([128, PT], F32, tag="pre", bufs=3, name="pre0")
            pre1 = psum.tile([128, PT], F32, tag="pre", bufs=3, name="pre1")
            nc.tensor.matmul(out=pre0[:, h0], lhsT=wt0[:, 0:128], rhs=ib0[:, h0], start=True, stop=False)
            nc.tensor.matmul(out=pre0[:, h1], lhsT=wt0[:, 0:128], rhs=ib0[:, h1], start=True, stop=False)
            nc.tensor.matmul(out=pre0[:, h0], lhsT=wt1[:, 0:128], rhs=ib1[:, h0], start=False, stop=True)
            nc.tensor.matmul(out=pre0[:, h1], lhsT=wt1[:, 0:128], rhs=ib1[:, h1], start=False, stop=True)
            nc.tensor.matmul(out=pre1[:, h0], lhsT=wt0[:, 128:256], rhs=ib0[:, h0], start=True, stop=False)
            nc.tensor.matmul(out=pre1[:, h1], lhsT=wt0[:, 128:256], rhs=ib0[:, h1], start=True, stop=False)
            nc.tensor.matmul(out=pre1[:, h0], lhsT=wt1[:, 128:256], rhs=ib1[:, h0], start=False, stop=True)
            nc.tensor.matmul(out=pre1[:, h1], lhsT=wt1[:, 128:256], rhs=ib1[:, h1], start=False, stop=True)
            pres[p] = (pre0, pre1)

        def emit_attn(p):
            cbs = cbss.pop(p)
            # the last attns run after the main matmuls are done, so they can
            # borrow freed "pre" psum banks -- avoids serializing the tail on
            # the single pz buffer
            if p >= NP - 2:
                pz = psum.tile([128, PT], F32, tag="pre", bufs=3, name="pzt")
            else:
                pz = psum.tile([128, PT], F32, tag="pz", bufs=1, name="pz")
            nc.tensor.matmul(out=pz[:, h0], lhsT=ones[:], rhs=cbs[:, h0], start=True, stop=True)
            nc.tensor.matmul(out=pz[:, h1], lhsT=ones[:], rhs=cbs[:, h1], start=True, stop=True)
            return pz

        def emit_tanh(p):
            pre0, pre1 = pres.pop(p)
            cb0 = combp.tile([128, PT], BF16, tag="cb0", name="cb0")
            cb1 = combp.tile([128, PT], BF16, tag="cb1", name="cb1")
            nc.scalar.activation(cb0[:], pre0[:], AF.Tanh, bias=b0[:, 0:1])
            nc.scalar.activation(cb1[:], pre1[:], AF.Tanh, bias=b1[:, 0:1])
            # cbs = a0*cb0 + a1*cb1  (per-partition scalars; z = ones^T cbs)
            cbt = cbsp.tile([128, PT], BF16, tag="cbt", bufs=2, name="cbt")
            nc.vector.tensor_scalar(cbt[:], cb0[:], a0c[:, 0:1], None, OP.mult)
            cbu = cbsp.tile([128, PT], BF16, tag="cbu", bufs=2, name="cbu")
            nc.vector.tensor_scalar(cbu[:], cb1[:], a1c[:, 0:1], None, OP.mult)
            cbs = cbsp.tile([128, PT], BF16, tag="cbs", bufs=4, name="cbs")
            nc.vector.tensor_tensor(cbs[:], cbt[:], cbu[:], op=OP.add)
            cbss[p] = cbs

        def emit_sigmoid(p, pz):
            sc = scorep.tile([128, PT], BF16, tag="sc", name="sc")
            nc.scalar.activation(sc[:], pz[:], AF.Sigmoid, bias=abt[:, 0:1])
            scs[p] = sc

        def emit_mul(pd):
            sc = scs.pop(pd)
            ib0, ib1 = ib_sl(pd)
            if pd % 2 == 0:
                o0 = outp.tile([128, 2 * PT], BF16, tag="oc0", name="o0")
                o1 = outp.tile([128, 2 * PT], BF16, tag="oc1", name="o1")
                oc0s.append(o0)
                oc1s.append(o1)
            wsl = slice((pd % 2) * PT, (pd % 2) * PT + PT)
            nc.vector.tensor_mul(oc0s[-1][:, wsl], ib0[:], sc[:])
            nc.vector.tensor_mul(oc1s[-1][:, wsl], ib1[:], sc[:])
            if pd >= NP - 2:
                # drain the last pairs as soon as each is multiplied
                ssl = bass.ts(pd, PT)
                nc.sync.dma_start(out[0:128, ssl], oc0s[-1][:, wsl])
                nc.sync.dma_start(out[128:256, ssl], oc1s[-1][:, wsl])
            elif pd % 2 == 1:
                ssl = bass.ts(pd // 2, 2 * PT)
                nc.sync.dma_start(out[0:128, ssl], oc0s[-1][:])
                nc.sync.dma_start(out[128:256, ssl], oc1s[-1][:])

        def fixup_part1():
            # pre-tanh + tanh + a-weighting for the <=19 keypoint columns.
            # Both 128-channel halves live in ONE pz-tag psum tile: cols
            # [0:19] in the first bank, [512:531] in the second, so each is
            # its own accumulation group and no extra psum bank is needed.
            pf = psum.tile([128, PT], F32, tag="pz", bufs=1, name="pf")
            kkB = slice(512, 512 + K)
            nc.tensor.matmul(out=pf[:, kk], lhsT=wt0[:, 0:128], rhs=ic0b[:], start=True, stop=False)
            nc.tensor.matmul(out=pf[:, kk], lhsT=wt1[:, 0:128], rhs=ic1b[:], start=False, stop=False)
            nc.tensor.matmul(out=pf[:, kk], lhsT=mts[:, 0:128], rhs=gbt[:], start=False, stop=True)
            nc.tensor.matmul(out=pf[:, kkB], lhsT=wt0[:, 128:256], rhs=ic0b[:], start=True, stop=False)
            nc.tensor.matmul(out=pf[:, kkB], lhsT=wt1[:, 128:256], rhs=ic1b[:], start=False, stop=False)
            nc.tensor.matmul(out=pf[:, kkB], lhsT=mts[:, 128:256], rhs=gbt[:], start=False, stop=True)
            cf0 = consts.tile([128, K], BF16)
            cf1 = consts.tile([128, K], BF16)
            nc.scalar.activation(cf0[:], pf[:, kk], AF.Tanh, bias=b0[:, 0:1])
            nc.scalar.activation(cf1[:], pf[:, kkB], AF.Tanh, bias=b1[:, 0:1])
            cft = consts.tile([128, K], BF16)
            nc.vector.tensor_scalar(cft[:], cf0[:], a0c[:, 0:1], None, OP.mult)
            cfs = consts.tile([128, K], BF16)
            nc.vector.scalar_tensor_tensor(
                cfs[:], cf1[:], a1c[:, 0:1], cft[:], op0=OP.mult, op1=OP.add)
            fix["cfs"] = cfs

        def fixup_part2():
            pzf = psum.tile([128, PT], F32, tag="pz", bufs=1, name="pzf")
            nc.tensor.matmul(out=pzf[:, kk], lhsT=ones[:], rhs=fix["cfs"][:], start=True, stop=True)
            scf = consts.tile([128, K], F32)
            nc.scalar.activation(scf[:], pzf[:, kk], AF.Sigmoid, bias=abt[:, 0:1])
            of0 = consts.tile([128, K], F32)
            of1 = consts.tile([128, K], F32)
            nc.vector.tensor_mul(of0[:], ic0b[:], scf[:])
            nc.vector.tensor_mul(of1[:], ic1b[:], scf[:])
            nc.sync.dma_start(ofix[0:128, :], of0[:])
            nc.sync.dma_start(ofix[128:256, :], of1[:])

        # PE warm-up: dummy matmuls on the weight tiles run while chunk0 is
        # still in flight, ramping the tensor engine out of its low p-state
        # (full clock needs ~3us of continuous execution) and priming FWL.
        for i in range(5):
            wp = psum.tile([128, PT], F32, tag="pre", bufs=3, name=f"wp{i}")
            nc.tensor.matmul(out=wp[:, 0:256], lhsT=wt0[:, 0:128], rhs=wt1[:], start=True, stop=True)
            nc.tensor.matmul(out=wp[:, 256:512], lhsT=wt1[:, 0:128], rhs=wt0[:], start=True, stop=True)

        # stage lags: main(p) / tanh(p-1) / attn+sigmoid(p-2) / mul+store(p-3)
        # -- ordered so every engine's queue head is ready (or nearly so) at
        # iteration start.
        CHUNK_AT = {0: 3, 4: 4, 8: 5}      # prefetch schedule (ch0-2 upfront)
        for p in range(NP + 3):
            if p in CHUNK_AT:
                load_chunk(CHUNK_AT[p])
            if p - 2 >= 0 and p - 2 < NP:
                pz = emit_attn(p - 2)
            if p < NP:
                emit_main(p)
            if p - 3 >= 0 and p - 3 < NP:
                emit_mul(p - 3)
            if p - 1 >= 0 and p - 1 < NP:
                emit_tanh(p - 1)
            if p - 2 >= 0 and p - 2 < NP:
                emit_sigmoid(p - 2, pz)
            # keypoint fixup interleaves with the tail iterations: its tiny
            # matmul/act/mul chain overlaps the final stores
            if p == NP:
                fixup_part1()
            elif p == NP + 1:
                fixup_part2()


def _build():
    if "nc" in _CACHE:
        return _CACHE["nc"]
    nc = bacc.Bacc("TRN2", target_bir_lowering=False, debug=False)
    io = {
        "img": nc.dram_tensor("img", [C, S], BF16, kind="ExternalInput").ap(),
        "imgcb": nc.dram_tensor("imgcb", [C, K], BF16, kind="ExternalInput").ap(),
        "gb": nc.dram_tensor("gb", [K, K], BF16, kind="ExternalInput").ap(),
        "wt": nc.dram_tensor("wt", [C, C], BF16, kind="ExternalInput").ap(),
        "mt": nc.dram_tensor("mt", [K, C], BF16, kind="ExternalInput").ap(),
        "bias": nc.dram_tensor("bias", [C, 1], F32, kind="ExternalInput").ap(),
        "acol": nc.dram_tensor("acol", [C, 1], F32, kind="ExternalInput").ap(),
        "ab": nc.dram_tensor("ab", [128, 1], F32, kind="ExternalInput").ap(),
        "out": nc.dram_tensor("out", [C, S], BF16, kind="ExternalOutput").ap(),
        "ofix": nc.dram_tensor("ofix", [C, K], F32, kind="ExternalOutput").ap(),
    }
    with tile.TileContext(nc) as tc:
        _emit(tc, io)
    nc.compile()
    _CACHE["nc"] = nc
    return nc


def _host_indices(keypoint_features):
    """Exact replication of the reference index math (all ops are exact in
    fp32: /128 is a power-of-two divide, clip, truncate)."""
    kps = np.asarray(keypoint_features, dtype=np.float32)        # [B, K, 3]
    x = np.clip(kps[:, :, 0] / np.float32(W), 0.0, W - 1).astype(np.int32)
    y = np.clip(kps[:, :, 1] / np.float32(H), 0.0, H - 1).astype(np.int32)
    s = y.astype(np.int64) * W + x                                # [B, K]
    vis = kps[:, :, 2] > 0                                        # [B, K]
    return s, vis


def _in_maps(image_features, keypoint_features, img_fc_w, img_fc_b,
             kp_proj_w, kp_proj_b, kp_fc_w, kp_fc_b, attn_fc_w, attn_fc_b):
    import ml_dtypes

    f = lambda a: np.ascontiguousarray(np.asarray(a, dtype=np.float32))
    bf = lambda a: np.ascontiguousarray(
        np.asarray(a, dtype=np.float32).astype(ml_dtypes.bfloat16))
    img_fc_w, img_fc_b = f(img_fc_w), f(img_fc_b)
    kp_proj_w, kp_proj_b = f(kp_proj_w), f(kp_proj_b)
    kp_fc_w, kp_fc_b = f(kp_fc_w), f(kp_fc_b)
    attn_fc_w, attn_fc_b = f(attn_fc_w), f(attn_fc_b)

    wt = bf(img_fc_w.T)                                         # [C, C]
    mt = bf((kp_fc_w @ kp_proj_w).T)                            # [K, C]
    bias = f((img_fc_b + kp_fc_w @ kp_proj_b + kp_fc_b).reshape(C, 1))
    acol = f(attn_fc_w.reshape(C, 1))
    ab = np.full((128, 1), float(attn_fc_b.reshape(-1)[0]), np.float32)

    imgs = f(image_features).reshape(B, C, S)
    s, vis = _host_indices(keypoint_features)
    maps = []
    for b in range(B):
        g = (s[b][None, :] == s[b][:, None]) & vis[b][:, None]  # [j', j]
        imgc = np.ascontiguousarray(imgs[b][:, s[b]])           # [C, K]
        maps.append({
            "img": bf(imgs[b]),
            "imgcb": bf(imgc),
            "gb": bf(g.astype(np.float32)),
            "wt": wt, "mt": mt, "bias": bias, "acol": acol, "ab": ab,
        })
    return maps


def _run(in_maps, trace=False, tmpdir=None):
    nc = _build()
    return run_bass_kernel_spmd(
        nc, in_maps, core_ids=list(range(B)), trace=trace, tmpdir=tmpdir
    )


def _assemble(res, keypoint_features):
    s, _ = _host_indices(keypoint_features)
    outs = []
    for b in range(B):
        o = np.asarray(res.results[b]["out"]).astype(np.float32)  # [C, S]
        o[:, s[b]] = np.asarray(res.results[b]["ofix"])           # fixup cols
        outs.append(o.reshape(C, H, W))
    return np.stack(outs)


def kernel(**inputs) -> np.ndarray:
    res = _run(_in_maps(**inputs))
    return _assemble(res, inputs["keypoint_features"])


def _enable_axon_ntff_hook():
    """Recreate the missing antenv.axon_hooks module and register the NTFF
    profile hook (what trn_boot would do if the image shipped axon_hooks).
    Local profiling only; kernel() never calls this."""
    import types

    if "antenv.axon_hooks" in sys.modules:
        return
    mod = types.ModuleType("antenv.axon_hooks")
    state = {"hook": None}
    mod.set_axon_ntff_profile_hook = lambda h: state.__setitem__("hook", h)
    mod.get_axon_ntff_profile_hook = lambda: state["hook"]
    sys.modules["antenv.axon_hooks"] = mod
    import antenv

    antenv.axon_hooks = mod
    from trn_agent_boot.trn_boot import _ntff_profile_via_ctypes

    mod.set_axon_ntff_profile_hook(_ntff_profile_via_ctypes("/opt/axon/libaxon_pjrt.so"))
    # keep artifacts local -- no bucket in this container
    import concourse.bass_utils as bu

    bu.upload_artifacts = lambda tmpdir: tmpdir


def kernel_traced(**inputs):
    """Like kernel() but profiles: returns (out, exec_time_ns, tmpdir)."""
    import tempfile

    _enable_axon_ntff_hook()
    tmpdir = tempfile.mkdtemp(prefix="bass_trace_")
    res = _run(_in_maps(**inputs), trace=True, tmpdir=tmpdir)
    out = _assemble(res, inputs["keypoint_features"])
    return out, res.exec_time_ns, tmpdir


# revision 33
# speedup vs baseline: 1.2325x; 1.0525x over previous
"""Trainium2 Bass kernel for nn_AttentionLayer (scatter_memory).

Reference math (per batch b):
    heatmap[k,y,x] += vis_k at (y_k, x_k)              # scatter, <=19 nonzero px
    kp_feat = conv1x1_K->K(heatmap)                    # kp_proj_w/b
    img_proj = img_fc(img)                             # C x C linear over pixels
    kp_proj  = kp_fc(kp_feat)                          # K -> C linear
    combined = tanh(img_proj + kp_proj)
    scores   = sigmoid(attn_fc(combined))              # per-pixel scalar
    out      = img * scores

The keypoint path only perturbs the <=19 pixel columns hit by a keypoint:
    pre[o,s] = W img[:,s] + b_total + sum_{j: s_j == s} vis_j M[:,j]
with W = img_fc_w, M = kp_fc_w @ kp_proj_w, b_total folded on host.  The
device computes the DENSE no-keypoint path for all 16384 pixels, plus a tiny
19-column "fixup" using host-gathered image columns and a host-built [19,19]
collision matrix G[j',j] = vis_j' * (s_j' == s_j); the host overwrites those
<=19 columns of the returned image with the fixup columns (index math on host
is exact: /128 is a power-of-two divide).

Memory regime: all image I/O is bf16 (host casts in, host upcasts out), which
halves HBM traffic to ~16.8 MB/core (~47 us at 358 GB/s/NC).  Image loads and
output stores all ride the sync HWDGE ring (ordered so small constants and the
growing-size first chunks land first), leaving the Act engine to run
activations only -- Act is the steady-state pacer at 3 wide [128,1024] ops
(2 tanh + 1 sigmoid ~= 3.0 us) per 1024-px step.

Software pipeline (per 1024-px step p), each engine's queue head ready at
iteration start:
  PE : attn-reduce(p-2) [2 ones-matmuls], then 8 main matmuls(p)
       (plus ~10 warm-up matmuls pre-loop to ramp the PE p-state while the
       first image chunk is in flight)
  Act: tanh x2 (p-1) over 2-bank PSUM tiles, then sigmoid(p-2)
  DVE: scores-multiply x2 (p-3), then the a*tanh weighted-sum (p-1)
The attention z = sum_c a_c * comb[c,s] is computed as two per-partition-scalar
DVE multiplies (a lives in a [128,1] column) + add, then a ones-weights matmul
whose PSUM result is already broadcast across all 128 partitions, so sigmoid
and the final multiply need no partition-broadcast step.  The keypoint fixup
is emitted into the pipeline-drain iterations where all engines have slack.

Sharding: pure data parallelism, batch b -> NeuronCore b (weights replicated).
"""

import sys
from collections import deque
from contextlib import ExitStack

import numpy as np

sys.path.insert(0, "/opt/trn_rl_repo")

import concourse.bacc as bacc
import concourse.bass as bass
import concourse.mybir as mybir
import concourse.tile as tile
from concourse.bass_utils import run_bass_kernel_spmd

F32 = mybir.dt.float32
BF16 = mybir.dt.bfloat16
AF = mybir.ActivationFunctionType
OP = mybir.AluOpType

B, C, H, W, K = 8, 256, 128, 128, 19
S = H * W                  # 16384 pixels
PT = 1024                  # pixels per pipeline step (2 PSUM banks of f32)
NP = S // PT               # 16 steps
CH = 4096                  # pixels per DMA chunk (1 MB bf16 per half)
PPC = CH // PT             # 4 steps per chunk
NCH = S // CH              # 4 chunks
_CACHE: dict = {}


def _emit(tc: tile.TileContext, io: dict):
    nc = tc.nc
    img, imgcb2, gb, wt, mt, smalls, out, ofix = (
        io["img"], io["imgcb2"], io["gb"], io["wt"], io["mt"],
        io["smalls"], io["out"], io["ofix"],
    )
    with ExitStack() as ctx:
        consts = ctx.enter_context(tc.tile_pool(name="consts", bufs=1))
        imgp = ctx.enter_context(tc.tile_pool(name="imgp", bufs=1))
        outp = ctx.enter_context(tc.tile_pool(name="outp", bufs=3))
        combp = ctx.enter_context(tc.tile_pool(name="combp", bufs=2))
        cbsp = ctx.enter_context(tc.tile_pool(name="cbsp", bufs=3))
        scorep = ctx.enter_context(tc.tile_pool(name="scorep", bufs=2))
        psum = ctx.enter_context(tc.tile_pool(name="psum", bufs=1, space="PSUM"))

        # chunk layout: small chunks first so compute starts early, then
        # 4096-px chunks for DMA efficiency
        CHOFF = [0, 1024, 2048, 4096, 8192, 12288]
        CHSZ = [1024, 1024, 2048, 4096, 4096, 4096]
        P2C = [0, 1, 2, 2] + [3] * 4 + [4] * 4 + [5] * 4   # pair -> chunk
        im0s, im1s, oc0s, oc1s = [], [], [], []

        def load_chunk(c):
            im0 = imgp.tile([128, CHSZ[c]], BF16, tag=f"im0_{c}", name=f"im0_{c}")
            im1 = imgp.tile([128, CHSZ[c]], BF16, tag=f"im1_{c}", name=f"im1_{c}")
            csl = slice(CHOFF[c], CHOFF[c] + CHSZ[c])
            nc.sync.dma_start(im0[:], img[0:128, csl])
            nc.sync.dma_start(im1[:], img[128:256, csl])
            im0s.append(im0)
            im1s.append(im1)

        # ---- constants into SBUF, ordered so chunk0 lands ASAP; all the
        # tiny per-partition scalars ride in ONE [128,5] DMA (each dma_start
        # costs the sync engine ~600ns of issue time, which delays the image
        # chunks behind it in the FIFO) ----
        wt0 = consts.tile([128, C], BF16)          # W^T rows c=0..127
        wt1 = consts.tile([128, C], BF16)          # W^T rows c=128..255
        nc.sync.dma_start(wt0[:], wt[0:128, :])
        nc.sync.dma_start(wt1[:], wt[128:256, :])
        load_chunk(0)
        smt = consts.tile([128, 5], F32)           # b0|b1|ab|a0|a1 columns
        nc.sync.dma_start(smt[:], smalls[:, :])
        b0, b1 = smt[:, 0:1], smt[:, 1:2]
        abt = smt[:, 2:3]
        a0c, a1c = smt[:, 3:4], smt[:, 4:5]
        ones = consts.tile([128, 128], BF16)       # partition-sum stationary
        nc.vector.memset(ones[:], 1.0)
        load_chunk(1)
        load_chunk(2)
        # fixup constants (consumed by the tail fixup)
        mts = consts.tile([K, C], BF16)            # M^T [19, 256]
        nc.sync.dma_start(mts[:], mt[:, :])
        gbt = consts.tile([K, K], BF16)            # collision matrix
        nc.sync.dma_start(gbt[:], gb[:, :])
        icb = consts.tile([128, 2 * K], BF16)      # img cols, both halves
        nc.sync.dma_start(icb[:], imgcb2[:, :])
        ic0b, ic1b = icb[:, 0:K], icb[:, K:2 * K]

        h0, h1 = bass.ts(0, 512), bass.ts(1, 512)
        kk = bass.ts(0, K)
        pres, cbss, scs = {}, {}, {}
        fix = {}                   # keypoint-fixup tiles, built mid-loop

        def ib_sl(pd):
            c = P2C[pd]
            off = pd * PT - CHOFF[c]
            return im0s[c][:, off:off + PT], im1s[c][:, off:off + PT]

        def emit_main(p):
            ib0, ib1 = ib_sl(p)
            pre0 = psum.tile([128, PT], F32, tag="pre", bufs=3, name="pre0")
            pre1 = psum.tile([128, PT], F32, tag="pre", bufs=3, name="pre1")
            nc.tensor.matmul(out=pre0[:, h0], lhsT=wt0[:, 0:128], rhs=ib0[:, h0], start=True, stop=False)
            nc.tensor.matmul(out=pre0[:, h1], lhsT=wt0[:, 0:128], rhs=ib0[:, h1], start=True, stop=False)
            nc.tensor.matmul(out=pre0[:, h0], lhsT=wt1[:, 0:128], rhs=ib1[:, h0], start=False, stop=True)
            nc.tensor.matmul(out=pre0[:, h1], lhsT=wt1[:, 0:128], rhs=ib1[:, h1], start=False, stop=True)
            nc.tensor.matmul(out=pre1[:, h0], lhsT=wt0[:, 128:256], rhs=ib0[:, h0], start=True, stop=False)
            nc.tensor.matmul(out=pre1[:, h1], lhsT=wt0[:, 128:256], rhs=ib0[:, h1], start=True, stop=False)
            nc.tensor.matmul(out=pre1[:, h0], lhsT=wt1[:, 128:256], rhs=ib1[:, h0], start=False, stop=True)
            nc.tensor.matmul(out=pre1[:, h1], lhsT=wt1[:, 128:256], rhs=ib1[:, h1], start=False, stop=True)
            pres[p] = (pre0, pre1)

        def emit_attn(p):
            cbs = cbss.pop(p)
            # the last attns run after the main matmuls are done, so they can
            # borrow freed "pre" psum banks -- avoids serializing the tail on
            # the single pz buffer
            if p >= NP - 2:
                pz = psum.tile([128, PT], F32, tag="pre", bufs=3, name="pzt")
            else:
                pz = psum.tile([128, PT], F32, tag="pz", bufs=1, name="pz")
            nc.tensor.matmul(out=pz[:, h0], lhsT=ones[:], rhs=cbs[:, h0], start=True, stop=True)
            nc.tensor.matmul(out=pz[:, h1], lhsT=ones[:], rhs=cbs[:, h1], start=True, stop=True)
            return pz

        def emit_tanh(p):
            pre0, pre1 = pres.pop(p)
            cb0 = combp.tile([128, PT], BF16, tag="cb0", name="cb0")
            cb1 = combp.tile([128, PT], BF16, tag="cb1", name="cb1")
            nc.scalar.activation(cb0[:], pre0[:], AF.Tanh, bias=b0)
            nc.scalar.activation(cb1[:], pre1[:], AF.Tanh, bias=b1)
            # cbs = a0*cb0 + a1*cb1  (per-partition scalars; z = ones^T cbs)
            cbt = cbsp.tile([128, PT], BF16, tag="cbt", bufs=2, name="cbt")
            nc.vector.tensor_scalar(cbt[:], cb0[:], a0c, None, OP.mult)
            cbu = cbsp.tile([128, PT], BF16, tag="cbu", bufs=2, name="cbu")
            nc.vector.tensor_scalar(cbu[:], cb1[:], a1c, None, OP.mult)
            cbs = cbsp.tile([128, PT], BF16, tag="cbs", bufs=4, name="cbs")
            nc.vector.tensor_tensor(cbs[:], cbt[:], cbu[:], op=OP.add)
            cbss[p] = cbs

        def emit_sigmoid(p, pz):
            sc = scorep.tile([128, PT], BF16, tag="sc", name="sc")
            nc.scalar.activation(sc[:], pz[:], AF.Sigmoid, bias=abt)
            scs[p] = sc

        def emit_mul(pd):
            sc = scs.pop(pd)
            ib0, ib1 = ib_sl(pd)
            if pd % 2 == 0:
                o0 = outp.tile([128, 2 * PT], BF16, tag="oc0", name="o0")
                o1 = outp.tile([128, 2 * PT], BF16, tag="oc1", name="o1")
                oc0s.append(o0)
                oc1s.append(o1)
            wsl = slice((pd % 2) * PT, (pd % 2) * PT + PT)
            nc.vector.tensor_mul(oc0s[-1][:, wsl], ib0[:], sc[:])
            nc.vector.tensor_mul(oc1s[-1][:, wsl], ib1[:], sc[:])
            if pd >= NP - 2:
                # drain the last pairs as soon as each is multiplied
                ssl = bass.ts(pd, PT)
                nc.sync.dma_start(out[0:128, ssl], oc0s[-1][:, wsl])
                nc.sync.dma_start(out[128:256, ssl], oc1s[-1][:, wsl])
            elif pd % 2 == 1:
                ssl = bass.ts(pd // 2, 2 * PT)
                nc.sync.dma_start(out[0:128, ssl], oc0s[-1][:])
                nc.sync.dma_start(out[128:256, ssl], oc1s[-1][:])

        def fixup_part1():
            # pre-tanh + tanh + a-weighting for the <=19 keypoint columns.
            # Both 128-channel halves live in ONE pz-tag psum tile: cols
            # [0:19] in the first bank, [512:531] in the second, so each is
            # its own accumulation group and no extra psum bank is needed.
            pf = psum.tile([128, PT], F32, tag="pz", bufs=1, name="pf")
            kkB = slice(512, 512 + K)
            nc.tensor.matmul(out=pf[:, kk], lhsT=wt0[:, 0:128], rhs=ic0b[:], start=True, stop=False)
            nc.tensor.matmul(out=pf[:, kk], lhsT=wt1[:, 0:128], rhs=ic1b[:], start=False, stop=False)
            nc.tensor.matmul(out=pf[:, kk], lhsT=mts[:, 0:128], rhs=gbt[:], start=False, stop=True)
            nc.tensor.matmul(out=pf[:, kkB], lhsT=wt0[:, 128:256], rhs=ic0b[:], start=True, stop=False)
            nc.tensor.matmul(out=pf[:, kkB], lhsT=wt1[:, 128:256], rhs=ic1b[:], start=False, stop=False)
            nc.tensor.matmul(out=pf[:, kkB], lhsT=mts[:, 128:256], rhs=gbt[:], start=False, stop=True)
            cf0 = consts.tile([128, K], BF16)
            cf1 = consts.tile([128, K], BF16)
            nc.scalar.activation(cf0[:], pf[:, kk], AF.Tanh, bias=b0)
            nc.scalar.activation(cf1[:], pf[:, kkB], AF.Tanh, bias=b1)
            cft = consts.tile([128, K], BF16)
            nc.vector.tensor_scalar(cft[:], cf0[:], a0c, None, OP.mult)
            cfs = consts.tile([128, K], BF16)
            nc.vector.scalar_tensor_tensor(
                cfs[:], cf1[:], a1c, cft[:], op0=OP.mult, op1=OP.add)
            fix["cfs"] = cfs

        def fixup_part2():
            pzf = psum.tile([128, PT], F32, tag="pz", bufs=1, name="pzf")
            nc.tensor.matmul(out=pzf[:, kk], lhsT=ones[:], rhs=fix["cfs"][:], start=True, stop=True)
            scf = consts.tile([128, K], F32)
            nc.scalar.activation(scf[:], pzf[:, kk], AF.Sigmoid, bias=abt)
            of0 = consts.tile([128, K], F32)
            of1 = consts.tile([128, K], F32)
            nc.vector.tensor_mul(of0[:], ic0b[:], scf[:])
            nc.vector.tensor_mul(of1[:], ic1b[:], scf[:])
            nc.sync.dma_start(ofix[0:128, :], of0[:])
            nc.sync.dma_start(ofix[128:256, :], of1[:])

        # PE warm-up: dummy matmuls on the weight tiles run while chunk0 is
        # still in flight, ramping the tensor engine out of its low p-state
        # (full clock needs ~3us of continuous execution) and priming FWL.
        for i in range(2):
            wp = psum.tile([128, PT], F32, tag="pre", bufs=3, name=f"wp{i}")
            nc.tensor.matmul(out=wp[:, 0:256], lhsT=wt0[:, 0:128], rhs=wt1[:], start=True, stop=True)
            nc.tensor.matmul(out=wp[:, 256:512], lhsT=wt1[:, 0:128], rhs=wt0[:], start=True, stop=True)

        # stage lags: main(p) / tanh(p-1) / attn+sigmoid(p-2) / mul+store(p-3)
        # -- ordered so every engine's queue head is ready (or nearly so) at
        # iteration start.
        CHUNK_AT = {0: 3, 4: 4, 8: 5}      # prefetch schedule (ch0-2 upfront)
        for p in range(NP + 3):
            if p in CHUNK_AT:
                load_chunk(CHUNK_AT[p])
            if p - 2 >= 0 and p - 2 < NP:
                pz = emit_attn(p - 2)
            if p < NP:
                emit_main(p)
            if p - 3 >= 0 and p - 3 < NP:
                emit_mul(p - 3)
            if p - 1 >= 0 and p - 1 < NP:
                emit_tanh(p - 1)
            if p - 2 >= 0 and p - 2 < NP:
                emit_sigmoid(p - 2, pz)
            # keypoint fixup interleaves with the tail iterations: its tiny
            # matmul/act/mul chain overlaps the final stores
            if p == NP:
                fixup_part1()
            elif p == NP + 1:
                fixup_part2()


def _build():
    if "nc" in _CACHE:
        return _CACHE["nc"]
    nc = bacc.Bacc("TRN2", target_bir_lowering=False, debug=False)
    io = {
        "img": nc.dram_tensor("img", [C, S], BF16, kind="ExternalInput").ap(),
        "imgcb2": nc.dram_tensor("imgcb2", [128, 2 * K], BF16, kind="ExternalInput").ap(),
        "gb": nc.dram_tensor("gb", [K, K], BF16, kind="ExternalInput").ap(),
        "wt": nc.dram_tensor("wt", [C, C], BF16, kind="ExternalInput").ap(),
        "mt": nc.dram_tensor("mt", [K, C], BF16, kind="ExternalInput").ap(),
        "smalls": nc.dram_tensor("smalls", [128, 5], F32, kind="ExternalInput").ap(),
        "out": nc.dram_tensor("out", [C, S], BF16, kind="ExternalOutput").ap(),
        "ofix": nc.dram_tensor("ofix", [C, K], F32, kind="ExternalOutput").ap(),
    }
    with tile.TileContext(nc) as tc:
        _emit(tc, io)
    nc.compile()
    _CACHE["nc"] = nc
    return nc


def _host_indices(keypoint_features):
    """Exact replication of the reference index math (all ops are exact in
    fp32: /128 is a power-of-two divide, clip, truncate)."""
    kps = np.asarray(keypoint_features, dtype=np.float32)        # [B, K, 3]
    x = np.clip(kps[:, :, 0] / np.float32(W), 0.0, W - 1).astype(np.int32)
    y = np.clip(kps[:, :, 1] / np.float32(H), 0.0, H - 1).astype(np.int32)
    s = y.astype(np.int64) * W + x                                # [B, K]
    vis = kps[:, :, 2] > 0                                        # [B, K]
    return s, vis


def _in_maps(image_features, keypoint_features, img_fc_w, img_fc_b,
             kp_proj_w, kp_proj_b, kp_fc_w, kp_fc_b, attn_fc_w, attn_fc_b):
    import ml_dtypes

    f = lambda a: np.ascontiguousarray(np.asarray(a, dtype=np.float32))
    bf = lambda a: np.ascontiguousarray(
        np.asarray(a, dtype=np.float32).astype(ml_dtypes.bfloat16))
    img_fc_w, img_fc_b = f(img_fc_w), f(img_fc_b)
    kp_proj_w, kp_proj_b = f(kp_proj_w), f(kp_proj_b)
    kp_fc_w, kp_fc_b = f(kp_fc_w), f(kp_fc_b)
    attn_fc_w, attn_fc_b = f(attn_fc_w), f(attn_fc_b)

    wt = bf(img_fc_w.T)                                         # [C, C]
    mt = bf((kp_fc_w @ kp_proj_w).T)                            # [K, C]
    bias = f(img_fc_b + kp_fc_w @ kp_proj_b + kp_fc_b)          # [C]
    acol = f(attn_fc_w.reshape(C))
    smalls = np.stack([
        bias[0:128], bias[128:256],
        np.full(128, float(attn_fc_b.reshape(-1)[0]), np.float32),
        acol[0:128], acol[128:256],
    ], axis=1).astype(np.float32)                               # [128, 5]
    smalls = np.ascontiguousarray(smalls)

    imgs = f(image_features).reshape(B, C, S)
    s, vis = _host_indices(keypoint_features)
    maps = []
    for b in range(B):
        g = (s[b][None, :] == s[b][:, None]) & vis[b][:, None]  # [j', j]
        imgc = imgs[b][:, s[b]]                                 # [C, K]
        imgc2 = np.concatenate([imgc[0:128], imgc[128:256]], axis=1)
        maps.append({
            "img": bf(imgs[b]),
            "imgcb2": bf(imgc2),
            "gb": bf(g.astype(np.float32)),
            "wt": wt, "mt": mt, "smalls": smalls,
        })
    return maps


def _run(in_maps, trace=False, tmpdir=None):
    nc = _build()
    return run_bass_kernel_spmd(
        nc, in_maps, core_ids=list(range(B)), trace=trace, tmpdir=tmpdir
    )


def _assemble(res, keypoint_features):
    s, _ = _host_indices(keypoint_features)
    outs = []
    for b in range(B):
        o = np.asarray(res.results[b]["out"]).astype(np.float32)  # [C, S]
        o[:, s[b]] = np.asarray(res.results[b]["ofix"])           # fixup cols
        outs.append(o.reshape(C, H, W))
    return np.stack(outs)


def kernel(**inputs) -> np.ndarray:
    res = _run(_in_maps(**inputs))
    return _assemble(res, inputs["keypoint_features"])


def _enable_axon_ntff_hook():
    """Recreate the missing antenv.axon_hooks module and register the NTFF
    profile hook (what trn_boot would do if the image shipped axon_hooks).
    Local profiling only; kernel() never calls this."""
    import types

    if "antenv.axon_hooks" in sys.modules:
        return
    mod = types.ModuleType("antenv.axon_hooks")
    state = {"hook": None}
    mod.set_axon_ntff_profile_hook = lambda h: state.__setitem__("hook", h)
    mod.get_axon_ntff_profile_hook = lambda: state["hook"]
    sys.modules["antenv.axon_hooks"] = mod
    import antenv

    antenv.axon_hooks = mod
    from trn_agent_boot.trn_boot import _ntff_profile_via_ctypes

    mod.set_axon_ntff_profile_hook(_ntff_profile_via_ctypes("/opt/axon/libaxon_pjrt.so"))
    # keep artifacts local -- no bucket in this container
    import concourse.bass_utils as bu

    bu.upload_artifacts = lambda tmpdir: tmpdir


def kernel_traced(**inputs):
    """Like kernel() but profiles: returns (out, exec_time_ns, tmpdir)."""
    import tempfile

    _enable_axon_ntff_hook()
    tmpdir = tempfile.mkdtemp(prefix="bass_trace_")
    res = _run(_in_maps(**inputs), trace=True, tmpdir=tmpdir)
    out = _assemble(res, inputs["keypoint_features"])
    return out, res.exec_time_ns, tmpdir


# revision 35
# speedup vs baseline: 1.2422x; 1.0079x over previous
"""Trainium2 Bass kernel for nn_AttentionLayer (scatter_memory).

Reference math (per batch b):
    heatmap[k,y,x] += vis_k at (y_k, x_k)              # scatter, <=19 nonzero px
    kp_feat = conv1x1_K->K(heatmap)                    # kp_proj_w/b
    img_proj = img_fc(img)                             # C x C linear over pixels
    kp_proj  = kp_fc(kp_feat)                          # K -> C linear
    combined = tanh(img_proj + kp_proj)
    scores   = sigmoid(attn_fc(combined))              # per-pixel scalar
    out      = img * scores

The keypoint path only perturbs the <=19 pixel columns hit by a keypoint:
    pre[o,s] = W img[:,s] + b_total + sum_{j: s_j == s} vis_j M[:,j]
with W = img_fc_w, M = kp_fc_w @ kp_proj_w, b_total folded on host.  The
device computes the DENSE no-keypoint path for all 16384 pixels, plus a tiny
19-column "fixup" using host-gathered image columns and a host-built [19,19]
collision matrix G[j',j] = vis_j' * (s_j' == s_j); the host overwrites those
<=19 columns of the returned image with the fixup columns (index math on host
is exact: /128 is a power-of-two divide).

Memory regime: all image I/O is bf16 (host casts in, host upcasts out), which
halves HBM traffic to ~16.8 MB/core (~47 us at 358 GB/s/NC).  Image loads and
output stores all ride the sync HWDGE ring (ordered so small constants and the
growing-size first chunks land first), leaving the Act engine to run
activations only -- Act is the steady-state pacer at 3 wide [128,1024] ops
(2 tanh + 1 sigmoid ~= 3.0 us) per 1024-px step.

Software pipeline (per 1024-px step p), each engine's queue head ready at
iteration start:
  PE : attn-reduce(p-2) [2 ones-matmuls], then 8 main matmuls(p)
       (plus ~10 warm-up matmuls pre-loop to ramp the PE p-state while the
       first image chunk is in flight)
  Act: tanh x2 (p-1) over 2-bank PSUM tiles, then sigmoid(p-2)
  DVE: scores-multiply x2 (p-3), then the a*tanh weighted-sum (p-1)
The attention z = sum_c a_c * comb[c,s] is computed as two per-partition-scalar
DVE multiplies (a lives in a [128,1] column) + add, then a ones-weights matmul
whose PSUM result is already broadcast across all 128 partitions, so sigmoid
and the final multiply need no partition-broadcast step.  The keypoint fixup
is emitted into the pipeline-drain iterations where all engines have slack.

Sharding: pure data parallelism, batch b -> NeuronCore b (weights replicated).
"""

import sys
from collections import deque
from contextlib import ExitStack

import numpy as np

sys.path.insert(0, "/opt/trn_rl_repo")

import concourse.bacc as bacc
import concourse.bass as bass
import concourse.mybir as mybir
import concourse.tile as tile
from concourse.bass_utils import run_bass_kernel_spmd

F32 = mybir.dt.float32
BF16 = mybir.dt.bfloat16
AF = mybir.ActivationFunctionType
OP = mybir.AluOpType

B, C, H, W, K = 8, 256, 128, 128, 19
S = H * W                  # 16384 pixels
PT = 1024                  # pixels per pipeline step (2 PSUM banks of f32)
NP = S // PT               # 16 steps
CH = 4096                  # pixels per DMA chunk (1 MB bf16 per half)
PPC = CH // PT             # 4 steps per chunk
NCH = S // CH              # 4 chunks
_CACHE: dict = {}


def _emit(tc: tile.TileContext, io: dict):
    nc = tc.nc
    img, imgcb2, gb, wt, mt, smalls, out, ofix = (
        io["img"], io["imgcb2"], io["gb"], io["wt"], io["mt"],
        io["smalls"], io["out"], io["ofix"],
    )
    with ExitStack() as ctx:
        consts = ctx.enter_context(tc.tile_pool(name="consts", bufs=1))
        imgp = ctx.enter_context(tc.tile_pool(name="imgp", bufs=1))
        outp = ctx.enter_context(tc.tile_pool(name="outp", bufs=3))
        combp = ctx.enter_context(tc.tile_pool(name="combp", bufs=2))
        cbsp = ctx.enter_context(tc.tile_pool(name="cbsp", bufs=3))
        scorep = ctx.enter_context(tc.tile_pool(name="scorep", bufs=2))
        psum = ctx.enter_context(tc.tile_pool(name="psum", bufs=1, space="PSUM"))

        # chunk layout: small chunks first so compute starts early, then
        # 4096-px chunks for DMA efficiency
        CHOFF = [0, 1024, 2048, 4096, 8192, 12288]
        CHSZ = [1024, 1024, 2048, 4096, 4096, 4096]
        P2C = [0, 1, 2, 2] + [3] * 4 + [4] * 4 + [5] * 4   # pair -> chunk
        im0s, im1s, oc0s, oc1s = [], [], [], []

        def load_chunk(c):
            im0 = imgp.tile([128, CHSZ[c]], BF16, tag=f"im0_{c}", name=f"im0_{c}")
            im1 = imgp.tile([128, CHSZ[c]], BF16, tag=f"im1_{c}", name=f"im1_{c}")
            csl = slice(CHOFF[c], CHOFF[c] + CHSZ[c])
            nc.sync.dma_start(im0[:], img[0:128, csl])
            nc.sync.dma_start(im1[:], img[128:256, csl])
            im0s.append(im0)
            im1s.append(im1)

        # ---- constants into SBUF, ordered so chunk0 lands ASAP; all the
        # tiny per-partition scalars ride in ONE [128,5] DMA (each dma_start
        # costs the sync engine ~600ns of issue time, which delays the image
        # chunks behind it in the FIFO) ----
        wt0 = consts.tile([128, C], BF16)          # W^T rows c=0..127
        wt1 = consts.tile([128, C], BF16)          # W^T rows c=128..255
        nc.sync.dma_start(wt0[:], wt[0:128, :])
        nc.sync.dma_start(wt1[:], wt[128:256, :])
        load_chunk(0)
        smt = consts.tile([128, 5], F32)           # b0|b1|ab|a0|a1 columns
        nc.sync.dma_start(smt[:], smalls[:, :])
        b0, b1 = smt[:, 0:1], smt[:, 1:2]
        abt = smt[:, 2:3]
        a0c, a1c = smt[:, 3:4], smt[:, 4:5]
        ones = consts.tile([128, 128], BF16)       # partition-sum stationary
        nc.vector.memset(ones[:], 1.0)
        load_chunk(1)
        load_chunk(2)
        load_chunk(3)
        # fixup constants (consumed by the tail fixup)
        mts = consts.tile([K, C], BF16)            # M^T [19, 256]
        nc.sync.dma_start(mts[:], mt[:, :])
        gbt = consts.tile([K, K], BF16)            # collision matrix
        nc.sync.dma_start(gbt[:], gb[:, :])
        icb = consts.tile([128, 2 * K], BF16)      # img cols, both halves
        nc.sync.dma_start(icb[:], imgcb2[:, :])
        ic0b, ic1b = icb[:, 0:K], icb[:, K:2 * K]

        h0, h1 = bass.ts(0, 512), bass.ts(1, 512)
        kk = bass.ts(0, K)
        pres, cbss, scs = {}, {}, {}
        fix = {}                   # keypoint-fixup tiles, built mid-loop

        def ib_sl(pd):
            c = P2C[pd]
            off = pd * PT - CHOFF[c]
            return im0s[c][:, off:off + PT], im1s[c][:, off:off + PT]

        def emit_main(p):
            ib0, ib1 = ib_sl(p)
            pre0 = psum.tile([128, PT], F32, tag="pre", bufs=3, name="pre0")
            pre1 = psum.tile([128, PT], F32, tag="pre", bufs=3, name="pre1")
            nc.tensor.matmul(out=pre0[:, h0], lhsT=wt0[:, 0:128], rhs=ib0[:, h0], start=True, stop=False)
            nc.tensor.matmul(out=pre0[:, h1], lhsT=wt0[:, 0:128], rhs=ib0[:, h1], start=True, stop=False)
            nc.tensor.matmul(out=pre0[:, h0], lhsT=wt1[:, 0:128], rhs=ib1[:, h0], start=False, stop=True)
            nc.tensor.matmul(out=pre0[:, h1], lhsT=wt1[:, 0:128], rhs=ib1[:, h1], start=False, stop=True)
            nc.tensor.matmul(out=pre1[:, h0], lhsT=wt0[:, 128:256], rhs=ib0[:, h0], start=True, stop=False)
            nc.tensor.matmul(out=pre1[:, h1], lhsT=wt0[:, 128:256], rhs=ib0[:, h1], start=True, stop=False)
            nc.tensor.matmul(out=pre1[:, h0], lhsT=wt1[:, 128:256], rhs=ib1[:, h0], start=False, stop=True)
            nc.tensor.matmul(out=pre1[:, h1], lhsT=wt1[:, 128:256], rhs=ib1[:, h1], start=False, stop=True)
            pres[p] = (pre0, pre1)

        def emit_attn(p):
            cbs = cbss.pop(p)
            # the last attns run after the main matmuls are done, so they can
            # borrow freed "pre" psum banks -- avoids serializing the tail on
            # the single pz buffer
            if p >= NP - 2:
                pz = psum.tile([128, PT], F32, tag="pre", bufs=3, name="pzt")
            else:
                pz = psum.tile([128, PT], F32, tag="pz", bufs=1, name="pz")
            nc.tensor.matmul(out=pz[:, h0], lhsT=ones[:], rhs=cbs[:, h0], start=True, stop=True)
            nc.tensor.matmul(out=pz[:, h1], lhsT=ones[:], rhs=cbs[:, h1], start=True, stop=True)
            return pz

        def emit_tanh(p):
            pre0, pre1 = pres.pop(p)
            cb0 = combp.tile([128, PT], BF16, tag="cb0", name="cb0")
            cb1 = combp.tile([128, PT], BF16, tag="cb1", name="cb1")
            nc.scalar.activation(cb0[:], pre0[:], AF.Tanh, bias=b0)
            nc.scalar.activation(cb1[:], pre1[:], AF.Tanh, bias=b1)
            # cbs = a0*cb0 + a1*cb1  (per-partition scalars; z = ones^T cbs)
            cbt = cbsp.tile([128, PT], BF16, tag="cbt", bufs=2, name="cbt")
            nc.vector.tensor_scalar(cbt[:], cb0[:], a0c, None, OP.mult)
            cbu = cbsp.tile([128, PT], BF16, tag="cbu", bufs=2, name="cbu")
            nc.vector.tensor_scalar(cbu[:], cb1[:], a1c, None, OP.mult)
            cbs = cbsp.tile([128, PT], BF16, tag="cbs", bufs=4, name="cbs")
            nc.vector.tensor_tensor(cbs[:], cbt[:], cbu[:], op=OP.add)
            cbss[p] = cbs

        def emit_sigmoid(p, pz):
            sc = scorep.tile([128, PT], BF16, tag="sc", name="sc")
            nc.scalar.activation(sc[:], pz[:], AF.Sigmoid, bias=abt)
            scs[p] = sc

        def emit_mul(pd):
            sc = scs.pop(pd)
            ib0, ib1 = ib_sl(pd)
            if pd % 2 == 0:
                o0 = outp.tile([128, 2 * PT], BF16, tag="oc0", name="o0")
                o1 = outp.tile([128, 2 * PT], BF16, tag="oc1", name="o1")
                oc0s.append(o0)
                oc1s.append(o1)
            wsl = slice((pd % 2) * PT, (pd % 2) * PT + PT)
            nc.vector.tensor_mul(oc0s[-1][:, wsl], ib0[:], sc[:])
            nc.vector.tensor_mul(oc1s[-1][:, wsl], ib1[:], sc[:])
            if pd >= NP - 2:
                # drain the last pairs as soon as each is multiplied
                ssl = bass.ts(pd, PT)
                nc.sync.dma_start(out[0:128, ssl], oc0s[-1][:, wsl])
                nc.sync.dma_start(out[128:256, ssl], oc1s[-1][:, wsl])
            elif pd % 2 == 1:
                ssl = bass.ts(pd // 2, 2 * PT)
                nc.sync.dma_start(out[0:128, ssl], oc0s[-1][:])
                nc.sync.dma_start(out[128:256, ssl], oc1s[-1][:])

        def fixup_part1():
            # pre-tanh + tanh + a-weighting for the <=19 keypoint columns.
            # Both 128-channel halves live in ONE pz-tag psum tile: cols
            # [0:19] in the first bank, [512:531] in the second, so each is
            # its own accumulation group and no extra psum bank is needed.
            pf = psum.tile([128, PT], F32, tag="pz", bufs=1, name="pf")
            kkB = slice(512, 512 + K)
            nc.tensor.matmul(out=pf[:, kk], lhsT=wt0[:, 0:128], rhs=ic0b[:], start=True, stop=False)
            nc.tensor.matmul(out=pf[:, kk], lhsT=wt1[:, 0:128], rhs=ic1b[:], start=False, stop=False)
            nc.tensor.matmul(out=pf[:, kk], lhsT=mts[:, 0:128], rhs=gbt[:], start=False, stop=True)
            nc.tensor.matmul(out=pf[:, kkB], lhsT=wt0[:, 128:256], rhs=ic0b[:], start=True, stop=False)
            nc.tensor.matmul(out=pf[:, kkB], lhsT=wt1[:, 128:256], rhs=ic1b[:], start=False, stop=False)
            nc.tensor.matmul(out=pf[:, kkB], lhsT=mts[:, 128:256], rhs=gbt[:], start=False, stop=True)
            cf0 = consts.tile([128, K], BF16)
            cf1 = consts.tile([128, K], BF16)
            nc.scalar.activation(cf0[:], pf[:, kk], AF.Tanh, bias=b0)
            nc.scalar.activation(cf1[:], pf[:, kkB], AF.Tanh, bias=b1)
            cft = consts.tile([128, K], BF16)
            nc.vector.tensor_scalar(cft[:], cf0[:], a0c, None, OP.mult)
            cfs = consts.tile([128, K], BF16)
            nc.vector.scalar_tensor_tensor(
                cfs[:], cf1[:], a1c, cft[:], op0=OP.mult, op1=OP.add)
            fix["cfs"] = cfs

        def fixup_part2():
            pzf = psum.tile([128, PT], F32, tag="pz", bufs=1, name="pzf")
            nc.tensor.matmul(out=pzf[:, kk], lhsT=ones[:], rhs=fix["cfs"][:], start=True, stop=True)
            scf = consts.tile([128, K], F32)
            nc.scalar.activation(scf[:], pzf[:, kk], AF.Sigmoid, bias=abt)
            of0 = consts.tile([128, K], F32)
            of1 = consts.tile([128, K], F32)
            nc.vector.tensor_mul(of0[:], ic0b[:], scf[:])
            nc.vector.tensor_mul(of1[:], ic1b[:], scf[:])
            nc.sync.dma_start(ofix[0:128, :], of0[:])
            nc.sync.dma_start(ofix[128:256, :], of1[:])

        # PE warm-up: dummy matmuls on the weight tiles run while chunk0 is
        # still in flight, ramping the tensor engine out of its low p-state
        # (full clock needs ~3us of continuous execution) and priming FWL.
        for i in range(2):
            wp = psum.tile([128, PT], F32, tag="pre", bufs=3, name=f"wp{i}")
            nc.tensor.matmul(out=wp[:, 0:256], lhsT=wt0[:, 0:128], rhs=wt1[:], start=True, stop=True)
            nc.tensor.matmul(out=wp[:, 256:512], lhsT=wt1[:, 0:128], rhs=wt0[:], start=True, stop=True)

        # stage lags: main(p) / tanh(p-1) / attn+sigmoid(p-2) / mul+store(p-3)
        # -- ordered so every engine's queue head is ready (or nearly so) at
        # iteration start.
        CHUNK_AT = {0: 4, 4: 5}            # prefetch schedule (ch0-3 upfront)
        for p in range(NP + 3):
            if p in CHUNK_AT:
                load_chunk(CHUNK_AT[p])
            if p - 2 >= 0 and p - 2 < NP:
                pz = emit_attn(p - 2)
            if p < NP:
                emit_main(p)
            if p - 3 >= 0 and p - 3 < NP:
                emit_mul(p - 3)
            if p - 1 >= 0 and p - 1 < NP:
                emit_tanh(p - 1)
            if p - 2 >= 0 and p - 2 < NP:
                emit_sigmoid(p - 2, pz)
            # keypoint fixup interleaves with the tail iterations: its tiny
            # matmul/act/mul chain overlaps the final stores
            if p == NP:
                fixup_part1()
            elif p == NP + 1:
                fixup_part2()


def _build():
    if "nc" in _CACHE:
        return _CACHE["nc"]
    nc = bacc.Bacc("TRN2", target_bir_lowering=False, debug=False)
    io = {
        "img": nc.dram_tensor("img", [C, S], BF16, kind="ExternalInput").ap(),
        "imgcb2": nc.dram_tensor("imgcb2", [128, 2 * K], BF16, kind="ExternalInput").ap(),
        "gb": nc.dram_tensor("gb", [K, K], BF16, kind="ExternalInput").ap(),
        "wt": nc.dram_tensor("wt", [C, C], BF16, kind="ExternalInput").ap(),
        "mt": nc.dram_tensor("mt", [K, C], BF16, kind="ExternalInput").ap(),
        "smalls": nc.dram_tensor("smalls", [128, 5], F32, kind="ExternalInput").ap(),
        "out": nc.dram_tensor("out", [C, S], BF16, kind="ExternalOutput").ap(),
        "ofix": nc.dram_tensor("ofix", [C, K], F32, kind="ExternalOutput").ap(),
    }
    with tile.TileContext(nc) as tc:
        _emit(tc, io)
    nc.compile()
    _CACHE["nc"] = nc
    return nc


def _host_indices(keypoint_features):
    """Exact replication of the reference index math (all ops are exact in
    fp32: /128 is a power-of-two divide, clip, truncate)."""
    kps = np.asarray(keypoint_features, dtype=np.float32)        # [B, K, 3]
    x = np.clip(kps[:, :, 0] / np.float32(W), 0.0, W - 1).astype(np.int32)
    y = np.clip(kps[:, :, 1] / np.float32(H), 0.0, H - 1).astype(np.int32)
    s = y.astype(np.int64) * W + x                                # [B, K]
    vis = kps[:, :, 2] > 0                                        # [B, K]
    return s, vis


def _in_maps(image_features, keypoint_features, img_fc_w, img_fc_b,
             kp_proj_w, kp_proj_b, kp_fc_w, kp_fc_b, attn_fc_w, attn_fc_b):
    import ml_dtypes

    f = lambda a: np.ascontiguousarray(np.asarray(a, dtype=np.float32))
    bf = lambda a: np.ascontiguousarray(
        np.asarray(a, dtype=np.float32).astype(ml_dtypes.bfloat16))
    img_fc_w, img_fc_b = f(img_fc_w), f(img_fc_b)
    kp_proj_w, kp_proj_b = f(kp_proj_w), f(kp_proj_b)
    kp_fc_w, kp_fc_b = f(kp_fc_w), f(kp_fc_b)
    attn_fc_w, attn_fc_b = f(attn_fc_w), f(attn_fc_b)

    wt = bf(img_fc_w.T)                                         # [C, C]
    mt = bf((kp_fc_w @ kp_proj_w).T)                            # [K, C]
    bias = f(img_fc_b + kp_fc_w @ kp_proj_b + kp_fc_b)          # [C]
    acol = f(attn_fc_w.reshape(C))
    smalls = np.stack([
        bias[0:128], bias[128:256],
        np.full(128, float(attn_fc_b.reshape(-1)[0]), np.float32),
        acol[0:128], acol[128:256],
    ], axis=1).astype(np.float32)                               # [128, 5]
    smalls = np.ascontiguousarray(smalls)

    imgs = f(image_features).reshape(B, C, S)
    s, vis = _host_indices(keypoint_features)
    maps = []
    for b in range(B):
        g = (s[b][None, :] == s[b][:, None]) & vis[b][:, None]  # [j', j]
        imgc = imgs[b][:, s[b]]                                 # [C, K]
        imgc2 = np.concatenate([imgc[0:128], imgc[128:256]], axis=1)
        maps.append({
            "img": bf(imgs[b]),
            "imgcb2": bf(imgc2),
            "gb": bf(g.astype(np.float32)),
            "wt": wt, "mt": mt, "smalls": smalls,
        })
    return maps


def _run(in_maps, trace=False, tmpdir=None):
    nc = _build()
    return run_bass_kernel_spmd(
        nc, in_maps, core_ids=list(range(B)), trace=trace, tmpdir=tmpdir
    )


def _assemble(res, keypoint_features):
    s, _ = _host_indices(keypoint_features)
    outs = []
    for b in range(B):
        o = np.asarray(res.results[b]["out"]).astype(np.float32)  # [C, S]
        o[:, s[b]] = np.asarray(res.results[b]["ofix"])           # fixup cols
        outs.append(o.reshape(C, H, W))
    return np.stack(outs)


def kernel(**inputs) -> np.ndarray:
    res = _run(_in_maps(**inputs))
    return _assemble(res, inputs["keypoint_features"])


def _enable_axon_ntff_hook():
    """Recreate the missing antenv.axon_hooks module and register the NTFF
    profile hook (what trn_boot would do if the image shipped axon_hooks).
    Local profiling only; kernel() never calls this."""
    import types

    if "antenv.axon_hooks" in sys.modules:
        return
    mod = types.ModuleType("antenv.axon_hooks")
    state = {"hook": None}
    mod.set_axon_ntff_profile_hook = lambda h: state.__setitem__("hook", h)
    mod.get_axon_ntff_profile_hook = lambda: state["hook"]
    sys.modules["antenv.axon_hooks"] = mod
    import antenv

    antenv.axon_hooks = mod
    from trn_agent_boot.trn_boot import _ntff_profile_via_ctypes

    mod.set_axon_ntff_profile_hook(_ntff_profile_via_ctypes("/opt/axon/libaxon_pjrt.so"))
    # keep artifacts local -- no bucket in this container
    import concourse.bass_utils as bu

    bu.upload_artifacts = lambda tmpdir: tmpdir


def kernel_traced(**inputs):
    """Like kernel() but profiles: returns (out, exec_time_ns, tmpdir)."""
    import tempfile

    _enable_axon_ntff_hook()
    tmpdir = tempfile.mkdtemp(prefix="bass_trace_")
    res = _run(_in_maps(**inputs), trace=True, tmpdir=tmpdir)
    out = _assemble(res, inputs["keypoint_features"])
    return out, res.exec_time_ns, tmpdir


# revision 36
# speedup vs baseline: 1.2433x; 1.0008x over previous
"""Trainium2 Bass kernel for nn_AttentionLayer (scatter_memory).

Reference math (per batch b):
    heatmap[k,y,x] += vis_k at (y_k, x_k)              # scatter, <=19 nonzero px
    kp_feat = conv1x1_K->K(heatmap)                    # kp_proj_w/b
    img_proj = img_fc(img)                             # C x C linear over pixels
    kp_proj  = kp_fc(kp_feat)                          # K -> C linear
    combined = tanh(img_proj + kp_proj)
    scores   = sigmoid(attn_fc(combined))              # per-pixel scalar
    out      = img * scores

The keypoint path only perturbs the <=19 pixel columns hit by a keypoint:
    pre[o,s] = W img[:,s] + b_total + sum_{j: s_j == s} vis_j M[:,j]
with W = img_fc_w, M = kp_fc_w @ kp_proj_w, b_total folded on host.  The
device computes the DENSE no-keypoint path for all 16384 pixels, plus a tiny
19-column "fixup" using host-gathered image columns and a host-built [19,19]
collision matrix G[j',j] = vis_j' * (s_j' == s_j); the host overwrites those
<=19 columns of the returned image with the fixup columns (index math on host
is exact: /128 is a power-of-two divide).

Memory regime: all image I/O is bf16 (host casts in, host upcasts out), which
halves HBM traffic to ~16.8 MB/core (~47 us at 358 GB/s/NC).  Image loads and
output stores all ride the sync HWDGE ring (ordered so small constants and the
growing-size first chunks land first), leaving the Act engine to run
activations only -- Act is the steady-state pacer at 3 wide [128,1024] ops
(2 tanh + 1 sigmoid ~= 3.0 us) per 1024-px step.

Software pipeline (per 1024-px step p), each engine's queue head ready at
iteration start:
  PE : attn-reduce(p-2) [2 ones-matmuls], then 8 main matmuls(p)
       (plus 4 warm-up matmuls pre-loop to ramp the PE p-state while the
       first image chunk is in flight)
  Act: tanh x2 (p-1) over 2-bank PSUM tiles, then sigmoid(p-2)
  DVE: scores-multiply x2 (p-3), then the a*tanh weighted-sum (p-1)
The attention z = sum_c a_c * comb[c,s] is computed as two per-partition-scalar
DVE multiplies (a lives in a [128,1] column) + add, then a ones-weights matmul
whose PSUM result is already broadcast across all 128 partitions, so sigmoid
and the final multiply need no partition-broadcast step.  The keypoint fixup
is emitted into the pipeline-drain iterations where all engines have slack.

Sharding: pure data parallelism, batch b -> NeuronCore b (weights replicated).
"""

import sys
from collections import deque
from contextlib import ExitStack

import numpy as np

sys.path.insert(0, "/opt/trn_rl_repo")

import concourse.bacc as bacc
import concourse.bass as bass
import concourse.mybir as mybir
import concourse.tile as tile
from concourse.bass_utils import run_bass_kernel_spmd

F32 = mybir.dt.float32
BF16 = mybir.dt.bfloat16
AF = mybir.ActivationFunctionType
OP = mybir.AluOpType

B, C, H, W, K = 8, 256, 128, 128, 19
S = H * W                  # 16384 pixels
PT = 1024                  # pixels per pipeline step (2 PSUM banks of f32)
NP = S // PT               # 16 steps
CH = 4096                  # pixels per DMA chunk (1 MB bf16 per half)
PPC = CH // PT             # 4 steps per chunk
NCH = S // CH              # 4 chunks
_CACHE: dict = {}


def _emit(tc: tile.TileContext, io: dict):
    nc = tc.nc
    img, imgcb2, gb, wt, mt, smalls, out, ofix = (
        io["img"], io["imgcb2"], io["gb"], io["wt"], io["mt"],
        io["smalls"], io["out"], io["ofix"],
    )
    with ExitStack() as ctx:
        consts = ctx.enter_context(tc.tile_pool(name="consts", bufs=1))
        imgp = ctx.enter_context(tc.tile_pool(name="imgp", bufs=1))
        outp = ctx.enter_context(tc.tile_pool(name="outp", bufs=3))
        combp = ctx.enter_context(tc.tile_pool(name="combp", bufs=2))
        cbsp = ctx.enter_context(tc.tile_pool(name="cbsp", bufs=3))
        scorep = ctx.enter_context(tc.tile_pool(name="scorep", bufs=2))
        psum = ctx.enter_context(tc.tile_pool(name="psum", bufs=1, space="PSUM"))

        # chunk layout: small chunks first so compute starts early, then
        # 4096-px chunks for DMA efficiency
        CHOFF = [0, 1024, 2048, 4096, 8192, 12288]
        CHSZ = [1024, 1024, 2048, 4096, 4096, 4096]
        P2C = [0, 1, 2, 2] + [3] * 4 + [4] * 4 + [5] * 4   # pair -> chunk
        im0s, im1s, oc0s, oc1s = [], [], [], []

        def load_chunk(c):
            im0 = imgp.tile([128, CHSZ[c]], BF16, tag=f"im0_{c}", name=f"im0_{c}")
            im1 = imgp.tile([128, CHSZ[c]], BF16, tag=f"im1_{c}", name=f"im1_{c}")
            csl = slice(CHOFF[c], CHOFF[c] + CHSZ[c])
            nc.sync.dma_start(im0[:], img[0:128, csl])
            nc.sync.dma_start(im1[:], img[128:256, csl])
            im0s.append(im0)
            im1s.append(im1)

        # ---- constants into SBUF, ordered so chunk0 lands ASAP; all the
        # tiny per-partition scalars ride in ONE [128,5] DMA (each dma_start
        # costs the sync engine ~600ns of issue time, which delays the image
        # chunks behind it in the FIFO) ----
        wt0 = consts.tile([128, C], BF16)          # W^T rows c=0..127
        wt1 = consts.tile([128, C], BF16)          # W^T rows c=128..255
        nc.sync.dma_start(wt0[:], wt[0:128, :])
        nc.sync.dma_start(wt1[:], wt[128:256, :])
        load_chunk(0)
        smt = consts.tile([128, 5], F32)           # b0|b1|ab|a0|a1 columns
        nc.sync.dma_start(smt[:], smalls[:, :])
        b0, b1 = smt[:, 0:1], smt[:, 1:2]
        abt = smt[:, 2:3]
        a0c, a1c = smt[:, 3:4], smt[:, 4:5]
        ones = consts.tile([128, 128], BF16)       # partition-sum stationary
        nc.vector.memset(ones[:], 1.0)
        load_chunk(1)
        load_chunk(2)
        load_chunk(3)
        # fixup constants (consumed by the tail fixup)
        mts = consts.tile([K, C], BF16)            # M^T [19, 256]
        nc.sync.dma_start(mts[:], mt[:, :])
        gbt = consts.tile([K, K], BF16)            # collision matrix
        nc.sync.dma_start(gbt[:], gb[:, :])
        icb = consts.tile([128, 2 * K], BF16)      # img cols, both halves
        nc.sync.dma_start(icb[:], imgcb2[:, :])
        ic0b, ic1b = icb[:, 0:K], icb[:, K:2 * K]

        h0, h1 = bass.ts(0, 512), bass.ts(1, 512)
        kk = bass.ts(0, K)
        pres, cbss, scs = {}, {}, {}
        fix = {}                   # keypoint-fixup tiles, built mid-loop

        def ib_sl(pd):
            c = P2C[pd]
            off = pd * PT - CHOFF[c]
            return im0s[c][:, off:off + PT], im1s[c][:, off:off + PT]

        def emit_main(p):
            ib0, ib1 = ib_sl(p)
            pre0 = psum.tile([128, PT], F32, tag="pre", bufs=3, name="pre0")
            pre1 = psum.tile([128, PT], F32, tag="pre", bufs=3, name="pre1")
            nc.tensor.matmul(out=pre0[:, h0], lhsT=wt0[:, 0:128], rhs=ib0[:, h0], start=True, stop=False)
            nc.tensor.matmul(out=pre0[:, h1], lhsT=wt0[:, 0:128], rhs=ib0[:, h1], start=True, stop=False)
            nc.tensor.matmul(out=pre0[:, h0], lhsT=wt1[:, 0:128], rhs=ib1[:, h0], start=False, stop=True)
            nc.tensor.matmul(out=pre0[:, h1], lhsT=wt1[:, 0:128], rhs=ib1[:, h1], start=False, stop=True)
            nc.tensor.matmul(out=pre1[:, h0], lhsT=wt0[:, 128:256], rhs=ib0[:, h0], start=True, stop=False)
            nc.tensor.matmul(out=pre1[:, h1], lhsT=wt0[:, 128:256], rhs=ib0[:, h1], start=True, stop=False)
            nc.tensor.matmul(out=pre1[:, h0], lhsT=wt1[:, 128:256], rhs=ib1[:, h0], start=False, stop=True)
            nc.tensor.matmul(out=pre1[:, h1], lhsT=wt1[:, 128:256], rhs=ib1[:, h1], start=False, stop=True)
            pres[p] = (pre0, pre1)

        def emit_attn(p):
            cbs = cbss.pop(p)
            # the last attns run after the main matmuls are done, so they can
            # borrow freed "pre" psum banks -- avoids serializing the tail on
            # the single pz buffer
            if p >= NP - 2:
                pz = psum.tile([128, PT], F32, tag="pre", bufs=3, name="pzt")
            else:
                pz = psum.tile([128, PT], F32, tag="pz", bufs=1, name="pz")
            nc.tensor.matmul(out=pz[:, h0], lhsT=ones[:], rhs=cbs[:, h0], start=True, stop=True)
            nc.tensor.matmul(out=pz[:, h1], lhsT=ones[:], rhs=cbs[:, h1], start=True, stop=True)
            return pz

        def emit_tanh(p):
            pre0, pre1 = pres.pop(p)
            cb0 = combp.tile([128, PT], BF16, tag="cb0", name="cb0")
            cb1 = combp.tile([128, PT], BF16, tag="cb1", name="cb1")
            nc.scalar.activation(cb0[:], pre0[:], AF.Tanh, bias=b0)
            nc.scalar.activation(cb1[:], pre1[:], AF.Tanh, bias=b1)
            # cbs = a0*cb0 + a1*cb1  (per-partition scalars; z = ones^T cbs)
            cbt = cbsp.tile([128, PT], BF16, tag="cbt", bufs=2, name="cbt")
            nc.vector.tensor_scalar(cbt[:], cb0[:], a0c, None, OP.mult)
            cbu = cbsp.tile([128, PT], BF16, tag="cbu", bufs=2, name="cbu")
            nc.vector.tensor_scalar(cbu[:], cb1[:], a1c, None, OP.mult)
            cbs = cbsp.tile([128, PT], BF16, tag="cbs", bufs=4, name="cbs")
            nc.vector.tensor_tensor(cbs[:], cbt[:], cbu[:], op=OP.add)
            cbss[p] = cbs

        def emit_sigmoid(p, pz):
            sc = scorep.tile([128, PT], BF16, tag="sc", name="sc")
            nc.scalar.activation(sc[:], pz[:], AF.Sigmoid, bias=abt)
            scs[p] = sc

        def emit_mul(pd):
            sc = scs.pop(pd)
            ib0, ib1 = ib_sl(pd)
            if pd % 2 == 0:
                o0 = outp.tile([128, 2 * PT], BF16, tag="oc0", name="o0")
                o1 = outp.tile([128, 2 * PT], BF16, tag="oc1", name="o1")
                oc0s.append(o0)
                oc1s.append(o1)
            wsl = slice((pd % 2) * PT, (pd % 2) * PT + PT)
            nc.vector.tensor_mul(oc0s[-1][:, wsl], ib0[:], sc[:])
            nc.vector.tensor_mul(oc1s[-1][:, wsl], ib1[:], sc[:])
            if pd >= NP - 2:
                # drain the last pairs as soon as each is multiplied
                ssl = bass.ts(pd, PT)
                nc.sync.dma_start(out[0:128, ssl], oc0s[-1][:, wsl])
                nc.sync.dma_start(out[128:256, ssl], oc1s[-1][:, wsl])
            elif pd % 2 == 1:
                ssl = bass.ts(pd // 2, 2 * PT)
                nc.sync.dma_start(out[0:128, ssl], oc0s[-1][:])
                nc.sync.dma_start(out[128:256, ssl], oc1s[-1][:])

        def fixup_part1():
            # pre-tanh + tanh + a-weighting for the <=19 keypoint columns.
            # Both 128-channel halves live in ONE pz-tag psum tile: cols
            # [0:19] in the first bank, [512:531] in the second, so each is
            # its own accumulation group and no extra psum bank is needed.
            pf = psum.tile([128, PT], F32, tag="pz", bufs=1, name="pf")
            kkB = slice(512, 512 + K)
            nc.tensor.matmul(out=pf[:, kk], lhsT=wt0[:, 0:128], rhs=ic0b[:], start=True, stop=False)
            nc.tensor.matmul(out=pf[:, kk], lhsT=wt1[:, 0:128], rhs=ic1b[:], start=False, stop=False)
            nc.tensor.matmul(out=pf[:, kk], lhsT=mts[:, 0:128], rhs=gbt[:], start=False, stop=True)
            nc.tensor.matmul(out=pf[:, kkB], lhsT=wt0[:, 128:256], rhs=ic0b[:], start=True, stop=False)
            nc.tensor.matmul(out=pf[:, kkB], lhsT=wt1[:, 128:256], rhs=ic1b[:], start=False, stop=False)
            nc.tensor.matmul(out=pf[:, kkB], lhsT=mts[:, 128:256], rhs=gbt[:], start=False, stop=True)
            cf0 = consts.tile([128, K], BF16)
            cf1 = consts.tile([128, K], BF16)
            nc.scalar.activation(cf0[:], pf[:, kk], AF.Tanh, bias=b0)
            nc.scalar.activation(cf1[:], pf[:, kkB], AF.Tanh, bias=b1)
            cft = consts.tile([128, K], BF16)
            nc.vector.tensor_scalar(cft[:], cf0[:], a0c, None, OP.mult)
            cfs = consts.tile([128, K], BF16)
            nc.vector.scalar_tensor_tensor(
                cfs[:], cf1[:], a1c, cft[:], op0=OP.mult, op1=OP.add)
            fix["cfs"] = cfs

        def fixup_part2():
            pzf = psum.tile([128, PT], F32, tag="pz", bufs=1, name="pzf")
            nc.tensor.matmul(out=pzf[:, kk], lhsT=ones[:], rhs=fix["cfs"][:], start=True, stop=True)
            scf = consts.tile([128, K], F32)
            nc.scalar.activation(scf[:], pzf[:, kk], AF.Sigmoid, bias=abt)
            of0 = consts.tile([128, K], F32)
            of1 = consts.tile([128, K], F32)
            nc.vector.tensor_mul(of0[:], ic0b[:], scf[:])
            nc.vector.tensor_mul(of1[:], ic1b[:], scf[:])
            nc.sync.dma_start(ofix[0:128, :], of0[:])
            nc.sync.dma_start(ofix[128:256, :], of1[:])

        # PE warm-up: dummy matmuls on the weight tiles run while chunk0 is
        # still in flight, ramping the tensor engine out of its low p-state
        # (full clock needs ~3us of continuous execution) and priming FWL.
        for i in range(2):
            wp = psum.tile([128, PT], F32, tag="pre", bufs=3, name=f"wp{i}")
            nc.tensor.matmul(out=wp[:, 0:256], lhsT=wt0[:, 0:128], rhs=wt1[:], start=True, stop=True)
            nc.tensor.matmul(out=wp[:, 256:512], lhsT=wt1[:, 0:128], rhs=wt0[:], start=True, stop=True)

        # stage lags: main(p) / tanh(p-1) / attn+sigmoid(p-2) / mul+store(p-3)
        # -- ordered so every engine's queue head is ready (or nearly so) at
        # iteration start.
        CHUNK_AT = {0: 4, 4: 5}            # prefetch schedule (ch0-3 upfront)
        for p in range(NP + 3):
            if p in CHUNK_AT:
                load_chunk(CHUNK_AT[p])
            if p - 2 >= 0 and p - 2 < NP:
                pz = emit_attn(p - 2)
            if p < NP:
                emit_main(p)
            if p - 3 >= 0 and p - 3 < NP:
                emit_mul(p - 3)
            if p - 1 >= 0 and p - 1 < NP:
                emit_tanh(p - 1)
            if p - 2 >= 0 and p - 2 < NP:
                emit_sigmoid(p - 2, pz)
            # keypoint fixup interleaves with the tail iterations: its tiny
            # matmul/act/mul chain overlaps the final stores
            if p == NP:
                fixup_part1()
            elif p == NP + 1:
                fixup_part2()


def _build():
    if "nc" in _CACHE:
        return _CACHE["nc"]
    nc = bacc.Bacc("TRN2", target_bir_lowering=False, debug=False)
    io = {
        "img": nc.dram_tensor("img", [C, S], BF16, kind="ExternalInput").ap(),
        "imgcb2": nc.dram_tensor("imgcb2", [128, 2 * K], BF16, kind="ExternalInput").ap(),
        "gb": nc.dram_tensor("gb", [K, K], BF16, kind="ExternalInput").ap(),
        "wt": nc.dram_tensor("wt", [C, C], BF16, kind="ExternalInput").ap(),
        "mt": nc.dram_tensor("mt", [K, C], BF16, kind="ExternalInput").ap(),
        "smalls": nc.dram_tensor("smalls", [128, 5], F32, kind="ExternalInput").ap(),
        "out": nc.dram_tensor("out", [C, S], BF16, kind="ExternalOutput").ap(),
        "ofix": nc.dram_tensor("ofix", [C, K], F32, kind="ExternalOutput").ap(),
    }
    with tile.TileContext(nc) as tc:
        _emit(tc, io)
    nc.compile()
    _CACHE["nc"] = nc
    return nc


def _host_indices(keypoint_features):
    """Exact replication of the reference index math (all ops are exact in
    fp32: /128 is a power-of-two divide, clip, truncate)."""
    kps = np.asarray(keypoint_features, dtype=np.float32)        # [B, K, 3]
    x = np.clip(kps[:, :, 0] / np.float32(W), 0.0, W - 1).astype(np.int32)
    y = np.clip(kps[:, :, 1] / np.float32(H), 0.0, H - 1).astype(np.int32)
    s = y.astype(np.int64) * W + x                                # [B, K]
    vis = kps[:, :, 2] > 0                                        # [B, K]
    return s, vis


def _in_maps(image_features, keypoint_features, img_fc_w, img_fc_b,
             kp_proj_w, kp_proj_b, kp_fc_w, kp_fc_b, attn_fc_w, attn_fc_b):
    import ml_dtypes

    f = lambda a: np.ascontiguousarray(np.asarray(a, dtype=np.float32))
    bf = lambda a: np.ascontiguousarray(
        np.asarray(a, dtype=np.float32).astype(ml_dtypes.bfloat16))
    img_fc_w, img_fc_b = f(img_fc_w), f(img_fc_b)
    kp_proj_w, kp_proj_b = f(kp_proj_w), f(kp_proj_b)
    kp_fc_w, kp_fc_b = f(kp_fc_w), f(kp_fc_b)
    attn_fc_w, attn_fc_b = f(attn_fc_w), f(attn_fc_b)

    wt = bf(img_fc_w.T)                                         # [C, C]
    mt = bf((kp_fc_w @ kp_proj_w).T)                            # [K, C]
    bias = f(img_fc_b + kp_fc_w @ kp_proj_b + kp_fc_b)          # [C]
    acol = f(attn_fc_w.reshape(C))
    smalls = np.stack([
        bias[0:128], bias[128:256],
        np.full(128, float(attn_fc_b.reshape(-1)[0]), np.float32),
        acol[0:128], acol[128:256],
    ], axis=1).astype(np.float32)                               # [128, 5]
    smalls = np.ascontiguousarray(smalls)

    imgs = f(image_features).reshape(B, C, S)
    s, vis = _host_indices(keypoint_features)
    maps = []
    for b in range(B):
        g = (s[b][None, :] == s[b][:, None]) & vis[b][:, None]  # [j', j]
        imgc = imgs[b][:, s[b]]                                 # [C, K]
        imgc2 = np.concatenate([imgc[0:128], imgc[128:256]], axis=1)
        maps.append({
            "img": bf(imgs[b]),
            "imgcb2": bf(imgc2),
            "gb": bf(g.astype(np.float32)),
            "wt": wt, "mt": mt, "smalls": smalls,
        })
    return maps


def _run(in_maps, trace=False, tmpdir=None):
    nc = _build()
    return run_bass_kernel_spmd(
        nc, in_maps, core_ids=list(range(B)), trace=trace, tmpdir=tmpdir
    )


def _assemble(res, keypoint_features):
    s, _ = _host_indices(keypoint_features)
    outs = []
    for b in range(B):
        o = np.asarray(res.results[b]["out"]).astype(np.float32)  # [C, S]
        o[:, s[b]] = np.asarray(res.results[b]["ofix"])           # fixup cols
        outs.append(o.reshape(C, H, W))
    return np.stack(outs)


def kernel(**inputs) -> np.ndarray:
    res = _run(_in_maps(**inputs))
    return _assemble(res, inputs["keypoint_features"])


def _enable_axon_ntff_hook():
    """Recreate the missing antenv.axon_hooks module and register the NTFF
    profile hook (what trn_boot would do if the image shipped axon_hooks).
    Local profiling only; kernel() never calls this."""
    import types

    if "antenv.axon_hooks" in sys.modules:
        return
    mod = types.ModuleType("antenv.axon_hooks")
    state = {"hook": None}
    mod.set_axon_ntff_profile_hook = lambda h: state.__setitem__("hook", h)
    mod.get_axon_ntff_profile_hook = lambda: state["hook"]
    sys.modules["antenv.axon_hooks"] = mod
    import antenv

    antenv.axon_hooks = mod
    from trn_agent_boot.trn_boot import _ntff_profile_via_ctypes

    mod.set_axon_ntff_profile_hook(_ntff_profile_via_ctypes("/opt/axon/libaxon_pjrt.so"))
    # keep artifacts local -- no bucket in this container
    import concourse.bass_utils as bu

    bu.upload_artifacts = lambda tmpdir: tmpdir


def kernel_traced(**inputs):
    """Like kernel() but profiles: returns (out, exec_time_ns, tmpdir)."""
    import tempfile

    _enable_axon_ntff_hook()
    tmpdir = tempfile.mkdtemp(prefix="bass_trace_")
    res = _run(_in_maps(**inputs), trace=True, tmpdir=tmpdir)
    out = _assemble(res, inputs["keypoint_features"])
    return out, res.exec_time_ns, tmpdir
